# revision 1
# baseline (speedup 1.0000x reference)
"""Discriminative loss kernel for Trainium2 (Bass/Tile), 8-core SPMD.

Data-parallel over batch: core b processes image b (B=8).
Per image the device computes, over P = 512*1024 pixels with D=8 channels
and K=5 instance labels (0 = background):
  pass 1 (flat [128, 4096] pixel layout):
      counts[k] = sum(label==k+1), sums[k,d] = sum_{label==k+1} e_d
      via tensor_scalar(is_equal) + tensor_tensor_reduce; cross-partition
      reduce via a PE ones-matmul.
  tiny device math: centers c = sums/max(counts,1), C2_k = |c_k|^2, and a
      block-diagonal stationary matrix holding -2*c for pass 2.
  pass 2 ((g,d) blocked layout: partition = g*8+d, g=16 pixel groups):
      psum[(g,k),f] = sum_d(-2 c_kd e_d) + |e|^2       (two PE matmuls)
      d = sqrt(psum + C2_k); h = relu(d - 0.5); h2 = h^2   (ACT)
      inst_sum[k] += sum_f h2 * (label==k+1)           (DVE TTR)
Host combines the per-image scalars into the final 4 losses.
"""

import os
import sys

import numpy as np

for _p in ("/opt/trn_rl_repo", "/root/.axon_site/_ro/trn_rl_repo"):
    if os.path.isdir(_p) and _p not in sys.path:
        sys.path.insert(0, _p)

import concourse.bass as bass
import concourse.tile as tile
from concourse import mybir
from concourse.bass_utils import run_bass_kernel_spmd

F32 = mybir.dt.float32
F32R = mybir.dt.float32r
BF16 = mybir.dt.bfloat16
Alu = mybir.AluOpType
Act = mybir.ActivationFunctionType

B, D, H, W = 8, 8, 512, 1024
P = H * W          # 524288 pixels
K = 5
R = 128            # sbuf partitions
COLS = P // R      # 4096
NCH = 16           # pass-1 load/cast chunks
CW = COLS // NCH   # 256
G = 16             # pass-2 pixel groups
GPP = P // G       # 32768 pixels per group
F = 1024           # pass-2 tile width
NT = GPP // F      # 32 tiles
DELTA_V = 0.5
DELTA_D = 3.0
ALPHA, BETA, GAMMA = 1.0, 1.0, 0.001


def _to_bf16(a):
    import ml_dtypes
    return a.astype(ml_dtypes.bfloat16)


def _build_consts():
    sel_cnt = np.zeros((R, 40), np.float32)
    sel_sum = np.zeros((R, 40), np.float32)
    for k in range(K):
        for d in range(D):
            sel_cnt[9 * k + 8, 8 * k + d] = 1.0
            sel_sum[9 * k + d, 8 * k + d] = 1.0
    sum5 = np.zeros((R, K), np.float32)
    for k in range(K):
        for d in range(D):
            sum5[8 * k + d, k] = 1.0
    rep80 = np.zeros((R, 80), np.float32)
    for g in range(G):
        for k in range(K):
            rep80[k, 5 * g + k] = 1.0
    smat = np.zeros((R, 80), np.float32)
    for kk in range(K):
        for d in range(D):
            for g in range(G):
                smat[8 * kk + d, 5 * g + kk] = 1.0
    dsel = np.zeros((R, R), np.float32)
    for k in range(K):
        for d in range(D):
            for g in range(G):
                dsel[8 * k + d, 8 * g + d] = 1.0
    blockmask = np.zeros((R, 80), np.float32)
    for g in range(G):
        for d in range(D):
            for k in range(K):
                blockmask[8 * g + d, 5 * g + k] = 1.0
    ones_col = np.ones((R, 1), np.float32)
    kpat = np.zeros((R, K), np.float32)
    kvec = np.zeros((R, 1), np.float32)
    for g in range(G):
        for k in range(K):
            kpat[5 * g + k, k] = 1.0
            kvec[5 * g + k, 0] = float(k + 1)
    return dict(sel_cnt=sel_cnt, sel_sum=sel_sum, sum5=sum5, rep80=rep80,
                smat=smat, dsel=dsel, blockmask=blockmask, ones_col=ones_col,
                kpat=kpat, kvec=kvec,
                blockmask_bf=_to_bf16(blockmask),
                kpat_bf=_to_bf16(kpat),
                ones_sq_bf=_to_bf16(np.ones((R, R), np.float32)))


def _ap(handle, offset, dims):
    return bass.AP(tensor=handle.tensor if isinstance(handle, bass.AP) else handle,
                   offset=offset, ap=[list(x) for x in dims])


def _split_multiwait(nc):
    """This container's walrus encodes at most one sync-wait per instruction;
    Tile's tail drain carries one wait per outstanding DMA queue. Hoist the
    extra waits onto single-wait drains inserted just before."""
    n_split = 0
    for blk in nc.m.functions[0].blocks:
        out = []
        changed = False
        for i in blk.instructions:
            si = i.sync_info
            if si is not None and len(si.on_wait) > 1:
                waits = list(si.on_wait)
                for w in waits[:-1]:
                    d = mybir.InstDrain(
                        name=nc.get_next_instruction_name(), ins=[], outs=[])
                    d.engine = i.engine
                    d.sync_info = mybir.SyncInfo(on_wait=[w], on_update=[])
                    out.append(d)
                    n_split += 1
                i.sync_info = mybir.SyncInfo(
                    on_wait=[waits[-1]], on_update=list(si.on_update))
                changed = True
            out.append(i)
        if changed:
            blk.instructions = out
    return n_split


def build_program():
    nc = bass.Bass()
    emb = nc.declare_dram_parameter("emb", [D, P], BF16, isOutput=False)
    maskb = nc.declare_dram_parameter("maskb", [P], BF16, isOutput=False)
    o_stats = nc.declare_dram_parameter("o_stats", [45], F32, isOutput=True)
    o_c = nc.declare_dram_parameter("o_c", [40], F32, isOutput=True)
    o_inst = nc.declare_dram_parameter("o_inst", [K], F32, isOutput=True)
    mbf = nc.dram_tensor("mbf", [K, P], BF16)

    cn = {k: nc.inline_tensor(v, name=f"c_{k}") for k, v in _build_consts().items()}

    with tile.TileContext(nc) as tc:
        with tc.tile_pool(name="singles", bufs=1) as singles, \
             tc.tile_pool(name="p1", bufs=2) as p1, \
             tc.tile_pool(name="mpool", bufs=4) as mpool, \
             tc.tile_pool(name="qpool", bufs=2) as qpool, \
             tc.tile_pool(name="p2a", bufs=8) as p2a, \
             tc.tile_pool(name="p2b", bufs=4) as p2b, \
             tc.tile_pool(name="psum_s", bufs=1, space="PSUM") as psum_s, \
             tc.tile_pool(name="psumR", bufs=2, space="PSUM") as psumR, \
             tc.tile_pool(name="psum2", bufs=2, space="PSUM") as psum2:

            # load constants
            sb = {}
            for name, h in cn.items():
                t = singles.tile(list(h.shape), h.dtype, tag=f"c_{name}")
                nc.sync.dma_start(out=t, in_=h[:])
                sb[name] = t

            # constants used as activation biases
            for cval in (0.0, -DELTA_V):
                ct = singles.tile([R, 1], F32, tag=f"bias_{cval}")
                nc.vector.memset(ct, cval)
                nc.const_aps.aps[(F32, cval)] = ct[:]

            NSPL = 2
            ebf_h = []
            lb_h = []
            for h_ in range(NSPL):
                te = singles.tile([R, D, COLS // NSPL], BF16, tag=f"ebf{h_}",
                                  name=f"ebf_h{h_}")
                tl = singles.tile([R, COLS // NSPL], BF16, tag=f"lb{h_}",
                                  name=f"lb_h{h_}")
                ebf_h.append(te)
                lb_h.append(tl)

            # ---------------- pass 1 ----------------
            # Two half-image rounds so segment-sum compute on half 0
            # overlaps DMA loads of half 1.
            accB90 = singles.tile([R, 45 * NSPL], F32)
            HALF = COLS // NSPL
            NJ = HALF // 512
            NCHH = NCH // NSPL
            for h_ in range(NSPL):
                base = h_ * HALF
                nc.sync.dma_start(
                    out=ebf_h[h_],
                    in_=_ap(emb, base, [[COLS, R], [P, D], [1, HALF]]))
                nc.sync.dma_start(
                    out=lb_h[h_], in_=_ap(maskb, base, [[COLS, R], [1, HALF]]))
                # per (k, d): bf16 product plane, PE column-sum into psum,
                # ACT copy-with-accumulate -> accB90 col (scale 1/128; the
                # later ones-matmul over 128 identical rows multiplies back)
                for k in range(K):
                    mk = mpool.tile([R, HALF], BF16, tag="mk")
                    nc.vector.tensor_scalar(
                        out=mk, in0=lb_h[h_], scalar1=float(k + 1),
                        scalar2=None, op0=Alu.is_equal)
                    nc.sync.dma_start(
                        out=_ap(mbf, k * P + base, [[COLS, R], [1, HALF]]),
                        in_=mk)
                    qpair = []
                    for dp in range(4):
                        qp = qpool.tile([R, 2, HALF], BF16, tag="q",
                                        name=f"q_{h_}_{k}_{dp}")
                        mk_b = bass.AP(tensor=mk.tensor, offset=mk.offset,
                                       ap=[list(mk.ap[0]), [0, 2],
                                           list(mk.ap[1])])
                        nc.vector.tensor_tensor(
                            out=qp, in0=ebf_h[h_][:, 2 * dp:2 * dp + 2, :],
                            in1=mk_b, op=Alu.mult)
                        qpair.append(qp)
                    for d in range(-1, D):
                        if d < 0:
                            plane = mk
                            col = 9 * k + 8
                        else:
                            plane = qpair[d // 2][:, d % 2, :]
                            col = 9 * k + d
                        ps = psumR.tile([R, 512], F32, tag="red")
                        for j in range(NJ):
                            nc.tensor.matmul(
                                ps, sb["ones_sq_bf"],
                                plane[:, j * 512:(j + 1) * 512],
                                start=(j == 0), stop=(j == NJ - 1))
                        junkA = mpool.tile([R, 512], F32, tag="junkA")
                        nc.scalar.activation(
                            out=junkA, in_=ps, func=Act.Copy, bias=0.0,
                            scale=1.0 / R,
                            accum_out=accB90[:, NSPL * col + h_:NSPL * col + h_ + 1])


            accB = singles.tile([R, 45], F32)
            nc.vector.tensor_reduce(
                out=accB, in_=accB90.rearrange("p (j h) -> p j h", h=NSPL),
                axis=mybir.AxisListType.X, op=Alu.add)

            # cross-partition: 128 identical rows x (stats/128) -> stats
            ps45 = psum_s.tile([45, 1], F32, tag="small")
            nc.tensor.matmul(ps45, accB, sb["ones_col"], start=True, stop=True)
            sb45 = singles.tile([R, 1], F32)
            nc.vector.memset(sb45, 0.0)
            nc.scalar.copy(out=sb45[:45, :], in_=ps45)
            nc.sync.dma_start(out=o_stats[:].unsqueeze(1), in_=sb45[:45, :])

            # ---------------- tiny math: centers ----------------
            ps40a = psum_s.tile([40, 1], F32, tag="small")
            nc.tensor.matmul(ps40a, sb["sel_cnt"], sb45, start=True, stop=True)
            ps40b = psum_s.tile([40, 1], F32, tag="small")
            nc.tensor.matmul(ps40b, sb["sel_sum"], sb45, start=True, stop=True)
            cntc = singles.tile([R, 1], F32)
            nc.vector.memset(cntc, 0.0)
            nc.vector.tensor_scalar(out=cntc[:40, :], in0=ps40a, scalar1=1.0,
                                    scalar2=None, op0=Alu.max)
            inv = singles.tile([R, 1], F32)
            nc.vector.memset(inv, 0.0)
            nc.vector.reciprocal(out=inv[:40, :], in_=cntc[:40, :])
            c40 = singles.tile([R, 1], F32)
            nc.vector.memset(c40, 0.0)
            nc.vector.tensor_tensor(out=c40[:40, :], in0=ps40b, in1=inv[:40, :],
                                    op=Alu.mult)
            nc.sync.dma_start(out=o_c[:].unsqueeze(1), in_=c40[:40, :])
            cm2 = singles.tile([R, 1], F32)
            nc.vector.memset(cm2, 0.0)
            nc.vector.tensor_scalar(out=cm2[:40, :], in0=c40[:40, :],
                                    scalar1=-2.0, scalar2=None, op0=Alu.mult)
            csq = singles.tile([R, 1], F32)
            nc.vector.memset(csq, 0.0)
            nc.vector.tensor_tensor(out=csq[:40, :], in0=c40[:40, :],
                                    in1=c40[:40, :], op=Alu.mult)
            ps5 = psum_s.tile([K, 1], F32, tag="small")
            nc.tensor.matmul(ps5, sb["sum5"], csq, start=True, stop=True)
            c2sb = singles.tile([R, 1], F32)
            nc.vector.memset(c2sb, 0.0)
            nc.scalar.copy(out=c2sb[:K, :], in_=ps5)
            ps80 = psum_s.tile([80, 1], F32, tag="small")
            nc.tensor.matmul(ps80, sb["rep80"], c2sb, start=True, stop=True)
            c2bias = singles.tile([R, 1], F32)
            nc.vector.memset(c2bias, 0.0)
            nc.scalar.copy(out=c2bias[:80, :], in_=ps80)

            # block-diagonal stationary: cblk[8g+d, 5g+k] = -2*c[k,d]
            rhsS = singles.tile([R, 80], F32)
            nc.vector.tensor_scalar(out=rhsS, in0=sb["smat"], scalar1=cm2,
                                    scalar2=None, op0=Alu.mult)
            psD = psum_s.tile([R, 80], F32, tag="small")
            nc.tensor.matmul(psD, sb["dsel"], rhsS, start=True, stop=True)
            cblk = singles.tile([R, 80], F32)
            nc.vector.tensor_tensor(out=cblk, in0=psD, in1=sb["blockmask"],
                                    op=Alu.mult)
            cblk_bf = singles.tile([R, 80], BF16)
            nc.vector.tensor_scalar(out=cblk_bf, in0=cblk, scalar1=1.0,
                                    scalar2=None, op0=Alu.mult)

            # ---------------- pass 2 ----------------
            psI2 = psum_s.tile([K, 512], F32, tag="inst")
            for t in range(NT):
                et2 = p2a.tile([R, F], BF16, tag="et2")
                nc.sync.dma_start(
                    out=et2, in_=_ap(emb, t * F, [[GPP, G], [P, D], [1, F]]))
                mm = p2a.tile([80, F], BF16, tag="mm")
                nc.sync.dma_start(
                    out=mm, in_=_ap(mbf, t * F, [[GPP, G], [P, K], [1, F]]))
                sq = p2a.tile([R, F], BF16, tag="sq")
                if t % 4 == 3:
                    nc.scalar.square(sq, et2)
                else:
                    nc.gpsimd.tensor_mul(sq, et2, et2)
                pt = psum2.tile([80, F], F32, tag="pt")
                for hh_ in range(2):
                    sl = slice(hh_ * 512, (hh_ + 1) * 512)
                    nc.tensor.matmul(pt[:, sl], cblk_bf, et2[:, sl],
                                     start=True, stop=False)
                    nc.tensor.matmul(pt[:, sl], sb["blockmask_bf"], sq[:, sl],
                                     start=False, stop=True)
                dd = p2b.tile([80, F], BF16, tag="dd")
                nc.scalar.activation(out=dd, in_=pt, func=Act.Sqrt,
                                     bias=c2bias[:80, :], scale=1.0)
                hh = p2b.tile([80, F], BF16, tag="hh")
                nc.vector.tensor_scalar(out=hh, in0=dd, scalar1=-DELTA_V,
                                        scalar2=0.0, op0=Alu.add, op1=Alu.max)
                h2 = p2b.tile([80, F], BF16, tag="h2")
                if t % 2 == 0:
                    nc.vector.tensor_tensor(out=h2, in0=hh, in1=hh,
                                            op=Alu.mult)
                else:
                    nc.scalar.square(h2, hh)
                q2 = p2b.tile([80, F], BF16, tag="q2")
                nc.vector.tensor_tensor(out=q2, in0=h2, in1=mm, op=Alu.mult)
                for hh2 in range(2):
                    sl = slice(hh2 * 512, (hh2 + 1) * 512)
                    nc.tensor.matmul(
                        psI2, sb["kpat_bf"][:80, :], q2[:, sl],
                        start=(t == 0 and hh2 == 0),
                        stop=(t == NT - 1 and hh2 == 1))

            junk5 = singles.tile([K, 512], F32)
            inst5 = singles.tile([K, 1], F32)
            nc.scalar.activation(out=junk5, in_=psI2, func=Act.Copy,
                                 bias=0.0, scale=1.0, accum_out=inst5)
            nc.sync.dma_start(out=o_inst[:].unsqueeze(1), in_=inst5)

    from concourse.library_overlay import lower_extended_insts
    lower_extended_insts(nc)
    _split_multiwait(nc)
    return nc


_NC_CACHE = None


def _get_nc():
    global _NC_CACHE
    if _NC_CACHE is None:
        _NC_CACHE = build_program()
    return _NC_CACHE


def run_device(embedding, maskf, trace=False):
    nc = _get_nc()
    in_maps = [
        {"emb": _to_bf16(np.ascontiguousarray(embedding[b].reshape(D, P))),
         "maskb": _to_bf16(np.ascontiguousarray(maskf[b].reshape(P)))}
        for b in range(B)
    ]
    res = run_bass_kernel_spmd(nc, in_maps, list(range(B)), trace=trace)
    return res


def finalize(per_core):
    """Combine per-image device stats into the 4 reference losses."""
    loss_var_b = np.zeros(B, np.float32)
    loss_dist_b = np.zeros(B, np.float32)
    loss_reg_b = np.zeros(B, np.float32)
    Ns = np.zeros(B, np.float32)
    iu = np.triu(np.ones((K, K), bool), k=1)
    for b in range(B):
        s45 = per_core[b]["o_stats"].astype(np.float32)
        c = per_core[b]["o_c"].astype(np.float32).reshape(K, D)
        inst = per_core[b]["o_inst"].astype(np.float32)
        counts = s45[8::9]
        present = counts > 0
        presentf = present.astype(np.float32)
        N = presentf.sum()
        Ns[b] = N
        inst_mean = inst / np.maximum(counts, 1.0)
        loss_var_b[b] = (inst_mean * presentf).sum() / max(N, 1.0)
        diff = c[:, None, :] - c[None, :, :]
        dist_sq = (diff ** 2).sum(-1)
        pair_mask = present[:, None] & present[None, :] & iu
        safe = np.sqrt(np.where(pair_mask, dist_sq, 1.0))
        term = np.maximum(2.0 * DELTA_D - safe, 0.0) ** 2 * pair_mask
        n_pairs = N * (N - 1.0) / 2.0
        loss_dist_b[b] = term.sum() / (n_pairs if N > 1 else 1.0)
        c_norm = np.sqrt(np.where(present, (c ** 2).sum(-1), 1.0))
        loss_reg_b[b] = (c_norm * presentf).sum() / max(N, 1.0)
    has = (Ns > 0).astype(np.float32)
    denom = max(has.sum(), 1.0)
    loss_var = float((loss_var_b * has).sum() / denom)
    loss_dist = float((loss_dist_b * has).sum() / denom)
    loss_reg = float((loss_reg_b * has).sum() / denom)
    total = ALPHA * loss_var + BETA * loss_dist + GAMMA * loss_reg
    return (np.float32(total), np.float32(loss_var),
            np.float32(loss_dist), np.float32(loss_reg))


def kernel(embedding, instance_mask):
    embedding = np.asarray(embedding, dtype=np.float32)
    maskf = np.asarray(instance_mask).astype(np.float32)
    res = run_device(embedding, maskf, trace=False)
    return finalize(res.results)



# revision 27
# speedup vs baseline: 1.6076x; 1.6076x over previous
"""Discriminative loss kernel for Trainium2 (Bass/Tile), 8-core SPMD.

Data-parallel over batch: core b processes image b (B=8).
Per image, P = 512*1024 pixels, D=8 channels, K=5 labels (0 = background).

pass 1 (segment sums on the PE via pixel-major chunks):
  The flat image [128, 8, 4096] is processed in 4 quarters. Each quarter:
  PE-transpose the 8 e-planes and the label plane into pixel-on-partition
  tiles ([128, 1024] per plane, via 128x128 transposing matmuls through
  PSUM), build one-hot planes ohT[k] with 4x tensor_scalar is_equal, then
  for each 128-pixel chunk accumulate
      psum[k, 0:8] += ohT_chunk^T @ eT_chunk,  psum[k, 8] += sum(ohT_chunk)
  into a single [5, 9] PSUM tile (stationary reloads are pipelined;
  4096 tiny matmuls total). This replaces the DVE masked-product planes,
  the ones-matmul reduction and the ACT accumulate pass of the previous
  version.
tiny device math: centers c = sums/counts, |c_k|^2 bias, and the
  block-diagonal stationary holding -2*c for pass 2 (as before).
pass 2 ((g,d) blocked layout: partition = g*8+d, g=16 pixel groups):
  psum[(g,k),f] = sum_d(-2 c_kd e_d) + |e|^2       (two PE matmuls)
  d = sqrt(psum + C2_k); h = relu(d - 0.5); h2 = h^2
  inst_sum[k] += sum_f h2 * (label==k+1)
  The one-hot arrives via a stride-0 broadcast DMA of the raw labels into
  [80, F] plus a 4x is_equal (no DRAM one-hot round trip).
Host combines the per-image scalars into the final 4 losses.
"""

import os
import sys

import numpy as np

for _p in ("/opt/trn_rl_repo", "/root/.axon_site/_ro/trn_rl_repo"):
    if os.path.isdir(_p) and _p not in sys.path:
        sys.path.insert(0, _p)

import concourse.bass as bass
import concourse.tile as tile
from concourse import mybir
from concourse.bass_utils import run_bass_kernel_spmd

F32 = mybir.dt.float32
BF16 = mybir.dt.bfloat16
Alu = mybir.AluOpType
Act = mybir.ActivationFunctionType

B, D, H, W = 8, 8, 512, 1024
P = H * W          # 524288 pixels
K = 5
R = 128            # sbuf partitions
COLS = P // R      # 4096 flat cols
NQ = 8             # pass-1 slices
QW = COLS // NQ    # 512 cols per slice
NB = QW // R       # 4 transpose blocks per plane per slice
G = 16             # pass-2 pixel groups
GPP = P // G       # 32768 pixels per group
F = 1024           # pass-2 tile width
NT = GPP // F      # 32 tiles
DELTA_V = 0.5
DELTA_D = 3.0
ALPHA, BETA, GAMMA = 1.0, 1.0, 0.001


def _to_bf16(a):
    import ml_dtypes
    return a.astype(ml_dtypes.bfloat16)


def _build_consts():
    sel_cnt = np.zeros((R, 40), np.float32)
    sel_sum = np.zeros((R, 40), np.float32)
    for k in range(K):
        for d in range(D):
            sel_cnt[9 * k + 8, 8 * k + d] = 1.0
            sel_sum[9 * k + d, 8 * k + d] = 1.0
    sum5 = np.zeros((R, K), np.float32)
    for k in range(K):
        for d in range(D):
            sum5[8 * k + d, k] = 1.0
    rep80 = np.zeros((R, 80), np.float32)
    for g in range(G):
        for k in range(K):
            rep80[k, 5 * g + k] = 1.0
    smat = np.zeros((R, 80), np.float32)
    for kk in range(K):
        for d in range(D):
            for g in range(G):
                smat[8 * kk + d, 5 * g + kk] = 1.0
    dsel = np.zeros((R, R), np.float32)
    for k in range(K):
        for d in range(D):
            for g in range(G):
                dsel[8 * k + d, 8 * g + d] = 1.0
    blockmask = np.zeros((R, 80), np.float32)
    for g in range(G):
        for d in range(D):
            for k in range(K):
                blockmask[8 * g + d, 5 * g + k] = 1.0
    ones_col = np.ones((R, 1), np.float32)
    kpat = np.zeros((R, K), np.float32)
    kvec = np.zeros((R, 1), np.float32)
    for g in range(G):
        for k in range(K):
            kpat[5 * g + k, k] = 1.0
            kvec[5 * g + k, 0] = float(k + 1)
    scat45 = np.zeros((R, 45), np.float32)
    dmask45 = np.zeros((R, 9), np.float32)
    for k in range(K):
        for dd in range(9):
            scat45[k, 9 * k + dd] = 1.0
            dmask45[9 * k + dd, dd] = 1.0
    f32_pack = dict(sel_cnt=sel_cnt, sel_sum=sel_sum, sum5=sum5,
                    rep80=rep80, smat=smat, dsel=dsel, blockmask=blockmask,
                    ones_col=ones_col, kpat=kpat, kvec=kvec,
                    scat45=scat45, dmask45=dmask45)
    bf_pack = dict(ident_bf=_to_bf16(np.eye(R, dtype=np.float32)),
                   blockmask_bf=_to_bf16(blockmask),
                   kpat_bf=_to_bf16(kpat))
    return f32_pack, bf_pack


def _ap(handle, offset, dims):
    return bass.AP(tensor=handle.tensor if isinstance(handle, bass.AP) else handle,
                   offset=offset, ap=[list(x) for x in dims])


def _tap(t, extra_offset, dims):
    """Sub-AP of an SBUF tile at t.offset + extra_offset."""
    return bass.AP(tensor=t.tensor, offset=t.offset + extra_offset,
                   ap=[list(x) for x in dims])


def _split_multiwait(nc):
    """This container's walrus encodes at most one sync-wait per instruction;
    Tile's tail drain carries one wait per outstanding DMA queue. Hoist the
    extra waits onto single-wait drains inserted just before."""
    n_split = 0
    for blk in nc.m.functions[0].blocks:
        out = []
        changed = False
        for i in blk.instructions:
            si = i.sync_info
            if si is not None and len(si.on_wait) > 1:
                waits = list(si.on_wait)
                for w in waits[:-1]:
                    d = mybir.InstDrain(
                        name=nc.get_next_instruction_name(), ins=[], outs=[])
                    d.engine = i.engine
                    d.sync_info = mybir.SyncInfo(on_wait=[w], on_update=[])
                    out.append(d)
                    n_split += 1
                i.sync_info = mybir.SyncInfo(
                    on_wait=[waits[-1]], on_update=list(si.on_update))
                changed = True
            out.append(i)
        if changed:
            blk.instructions = out
    return n_split


def build_program():
    nc = bass.Bass()
    emb = nc.declare_dram_parameter("emb", [D, P], BF16, isOutput=False)
    maskb = nc.declare_dram_parameter("maskb", [P], BF16, isOutput=False)
    o_stats = nc.declare_dram_parameter("o_stats", [45], F32, isOutput=True)
    o_c = nc.declare_dram_parameter("o_c", [40], F32, isOutput=True)
    o_inst = nc.declare_dram_parameter("o_inst", [K], F32, isOutput=True)

    f32_pack, bf_pack = _build_consts()
    cat_f32 = nc.inline_tensor(
        np.concatenate([v for v in f32_pack.values()], axis=1), name="c_f32")
    cat_bf = nc.inline_tensor(
        np.concatenate([v for v in bf_pack.values()], axis=1), name="c_bf")

    with tile.TileContext(nc) as tc:
        with tc.tile_pool(name="singles", bufs=1) as singles, \
             tc.tile_pool(name="ld", bufs=4) as ldp, \
             tc.tile_pool(name="tr", bufs=3) as trp, \
             tc.tile_pool(name="p2a", bufs=14) as p2a, \
             tc.tile_pool(name="p2l", bufs=6) as p2l, \
             tc.tile_pool(name="p2b", bufs=3) as p2b, \
             tc.tile_pool(name="psumA", bufs=2, space="PSUM") as psumA, \
             tc.tile_pool(name="psumB", bufs=1, space="PSUM") as psumB, \
             tc.tile_pool(name="psum2", bufs=2, space="PSUM") as psum2:

            # first data slice BEFORE the constants: the dozen constant
            # DMAs otherwise serialize ~13us of dead time at the head.
            ebf0 = ldp.tile([R, D, QW], BF16, tag="ebf", name="ebf0")
            lbq0 = ldp.tile([R, QW], BF16, tag="lbq", name="lb0")
            nc.sync.dma_start(
                out=ebf0, in_=_ap(emb, 0, [[COLS, R], [P, D], [1, QW]]))
            nc.sync.dma_start(
                out=lbq0, in_=_ap(maskb, 0, [[COLS, R], [1, QW]]))

            # load constants: two packed DMAs (bf16 first: transposes
            # need the identity), then carve slice views
            sb = {}
            wbf = sum(v.shape[1] for v in bf_pack.values())
            wf32 = sum(v.shape[1] for v in f32_pack.values())
            tbf = singles.tile([R, wbf], BF16, tag="c_bf")
            nc.sync.dma_start(out=tbf, in_=cat_bf[:])
            tf32 = singles.tile([R, wf32], F32, tag="c_f32")
            nc.sync.dma_start(out=tf32, in_=cat_f32[:])
            off = 0
            for name, v in bf_pack.items():
                sb[name] = tbf[:, off:off + v.shape[1]]
                off += v.shape[1]
            off = 0
            for name, v in f32_pack.items():
                sb[name] = tf32[:, off:off + v.shape[1]]
                off += v.shape[1]

            # constants used as activation biases
            for cval in (0.0, -DELTA_V):
                ct = singles.tile([R, 1], F32, tag=f"bias_{cval}")
                nc.vector.memset(ct, cval)
                nc.const_aps.aps[(F32, cval)] = ct[:]

            # ---------------- pass 1 ----------------
            # per quarter: transpose 8 e-planes + label plane to pixel-major,
            # one-hot via 4x is_equal, then 1024 accumulating chunk matmuls.
            ps59 = psumB.tile([K, D + 1], F32, tag="small")

            def p2_pre(t):
                """Center-independent pass-2 tile work (DMA, one-hot, |e|^2).
                Emitted inside pass 1 for the first NQ tiles to fill engine
                slack and shorten the pass-1 -> pass-2 transition."""
                et2 = p2a.tile([R, F], BF16, tag="et2", name=f"et2_{t}")
                nc.sync.dma_start(
                    out=et2, in_=_ap(emb, t * F, [[GPP, G], [P, D], [1, F]]))
                lb2 = p2l.tile([80, F], BF16, tag="lb2", name=f"lb2_{t}")
                nc.sync.dma_start(
                    out=lb2, in_=_ap(maskb, t * F, [[GPP, G], [0, K], [1, F]]))
                mm = p2a.tile([80, F], BF16, tag="mm", name=f"mm_{t}")
                nc.vector.tensor_scalar(
                    out=mm, in0=lb2, scalar1=sb["kvec"][:80, :],
                    scalar2=None, op0=Alu.is_equal)
                sq = p2a.tile([R, F], BF16, tag="sq", name=f"sq_{t}")
                if t % 8 in (0, 4):
                    nc.scalar.square(sq, et2)
                elif t % 8 in (2, 6):
                    nc.vector.tensor_tensor(out=sq, in0=et2, in1=et2,
                                            op=Alu.mult)
                else:
                    nc.gpsimd.tensor_mul(sq, et2, et2)
                return et2, mm, sq

            stage = {}
            pre2 = {}
            for q in range(NQ + 1):
                if q < NQ:
                    base = q * QW
                    if q == 0:
                        ebf, lbq = ebf0, lbq0
                    else:
                        ebf = ldp.tile([R, D, QW], BF16, tag="ebf",
                                       name=f"ebf{q}")
                        lbq = ldp.tile([R, QW], BF16, tag="lbq",
                                       name=f"lb{q}")
                        nc.sync.dma_start(
                            out=ebf,
                            in_=_ap(emb, base, [[COLS, R], [P, D], [1, QW]]))
                        nc.sync.dma_start(
                            out=lbq,
                            in_=_ap(maskb, base, [[COLS, R], [1, QW]]))

                    eT = trp.tile([R, D + 1, QW], BF16, tag="eT",
                                  name=f"eT{q}")
                    ohT = trp.tile([R, K, QW], BF16, tag="ohT",
                                   name=f"ohT{q}")
                    lT = trp.tile([R, QW], BF16, tag="lT", name=f"lT{q}")
                    if q < 3:
                        nc.gpsimd.memset(eT[:, D, :], 1.0)

                    # labels first (one-hots start while e transposes run)
                    pst = psumA.tile([R, QW], BF16, tag="trans")
                    for b in range(NB):
                        nc.tensor.transpose(
                            pst[:, b * R:(b + 1) * R],
                            lbq[:, b * R:(b + 1) * R], sb["ident_bf"])
                    nc.scalar.copy(out=lT, in_=pst)
                    for k in range(K):
                        nc.vector.tensor_scalar(
                            out=ohT[:, k, :], in0=lT, scalar1=float(k + 1),
                            scalar2=None, op0=Alu.is_equal)

                    for d in range(D):
                        pse = psumA.tile([R, QW], BF16, tag="trans",
                                         name=f"pse{q}_{d}")
                        for b in range(NB):
                            nc.tensor.transpose(
                                pse[:, b * R:(b + 1) * R],
                                ebf[:, d, b * R:(b + 1) * R], sb["ident_bf"])
                        # GPSIMD has no PSUM access: ACT/DVE only here
                        if d % 2 == 1:
                            nc.vector.tensor_copy(out=eT[:, d, :], in_=pse)
                        else:
                            nc.scalar.copy(out=eT[:, d, :], in_=pse)
                    stage[q] = (eT, ohT)
                    pre2[q] = p2_pre(q)

                if q >= 1:
                    # chunk matmuls for the PREVIOUS slice: PE runs these
                    # while this slice's copies drain on ACT/DVE/Pool.
                    eTp, ohTp = stage.pop(q - 1)
                    for c in range(QW):
                        nc.tensor.matmul(
                            ps59,
                            _tap(ohTp, c, [[ohTp.ap[0][0], R], [QW, K]]),
                            _tap(eTp, c, [[eTp.ap[0][0], R], [QW, D + 1]]),
                            start=(q == 1 and c == 0),
                            stop=(q == NQ and c == QW - 1))

            for t_ in range(NQ, NQ + 6):
                pre2[t_] = p2_pre(t_)

            # [5, 9] psum -> 45-long stats vector, fully on-chip:
            # scatter matmul to [45, 9], diagonal mask, free-dim reduce.
            # (a DMA rearrange here costs ~3us of all-engine stall)
            s59 = singles.tile([K, D + 1], F32)
            nc.scalar.copy(out=s59, in_=ps59)
            sb45 = singles.tile([R, 1], F32)
            nc.vector.memset(sb45, 0.0)
            ps45x9 = psumB.tile([45, 9], F32, tag="small")
            nc.tensor.matmul(ps45x9, sb["scat45"][:K, :], s59,
                             start=True, stop=True)
            s45x9 = singles.tile([45, 9], F32)
            nc.vector.tensor_tensor(out=s45x9, in0=ps45x9,
                                    in1=sb["dmask45"][:45, :], op=Alu.mult)
            nc.vector.tensor_reduce(
                out=sb45[:45, :], in_=s45x9,
                axis=mybir.AxisListType.X, op=Alu.add)
            nc.sync.dma_start(out=o_stats[:].unsqueeze(1), in_=sb45[:45, :])

            # ---------------- tiny math: centers ----------------
            ps40a = psumB.tile([40, 1], F32, tag="small")
            nc.tensor.matmul(ps40a, sb["sel_cnt"], sb45, start=True, stop=True)
            ps40b = psumB.tile([40, 1], F32, tag="small")
            nc.tensor.matmul(ps40b, sb["sel_sum"], sb45, start=True, stop=True)
            cntc = singles.tile([R, 1], F32)
            nc.vector.memset(cntc, 0.0)
            nc.vector.tensor_scalar(out=cntc[:40, :], in0=ps40a, scalar1=1.0,
                                    scalar2=None, op0=Alu.max)
            inv = singles.tile([R, 1], F32)
            nc.vector.memset(inv, 0.0)
            nc.vector.reciprocal(out=inv[:40, :], in_=cntc[:40, :])
            c40 = singles.tile([R, 1], F32)
            nc.vector.memset(c40, 0.0)
            nc.vector.tensor_tensor(out=c40[:40, :], in0=ps40b, in1=inv[:40, :],
                                    op=Alu.mult)
            nc.sync.dma_start(out=o_c[:].unsqueeze(1), in_=c40[:40, :])
            cm2 = singles.tile([R, 1], F32)
            nc.vector.memset(cm2, 0.0)
            nc.vector.tensor_scalar(out=cm2[:40, :], in0=c40[:40, :],
                                    scalar1=-2.0, scalar2=None, op0=Alu.mult)
            csq = singles.tile([R, 1], F32)
            nc.vector.memset(csq, 0.0)
            nc.vector.tensor_tensor(out=csq[:40, :], in0=c40[:40, :],
                                    in1=c40[:40, :], op=Alu.mult)
            ps5 = psumB.tile([K, 1], F32, tag="small")
            nc.tensor.matmul(ps5, sb["sum5"], csq, start=True, stop=True)
            c2sb = singles.tile([R, 1], F32)
            nc.vector.memset(c2sb, 0.0)
            nc.scalar.copy(out=c2sb[:K, :], in_=ps5)
            ps80 = psumB.tile([80, 1], F32, tag="small")
            nc.tensor.matmul(ps80, sb["rep80"], c2sb, start=True, stop=True)
            c2bias = singles.tile([R, 1], F32)
            nc.vector.memset(c2bias, 0.0)
            nc.scalar.copy(out=c2bias[:80, :], in_=ps80)

            # block-diagonal stationary: cblk[8g+d, 5g+k] = -2*c[k,d]
            rhsS = singles.tile([R, 80], F32)
            nc.vector.tensor_scalar(out=rhsS, in0=sb["smat"], scalar1=cm2,
                                    scalar2=None, op0=Alu.mult)
            psD = psumB.tile([R, 80], F32, tag="small")
            nc.tensor.matmul(psD, sb["dsel"], rhsS, start=True, stop=True)
            cblk = singles.tile([R, 80], F32)
            nc.vector.tensor_tensor(out=cblk, in0=psD, in1=sb["blockmask"],
                                    op=Alu.mult)
            cblk_bf = singles.tile([R, 80], BF16)
            nc.vector.tensor_scalar(out=cblk_bf, in0=cblk, scalar1=1.0,
                                    scalar2=None, op0=Alu.mult)

            # ---------------- pass 2 ----------------
            psI2 = psumB.tile([K, 512], F32, tag="small")
            for t in range(NT):
                if t in pre2:
                    et2, mm, sq = pre2.pop(t)
                else:
                    et2, mm, sq = p2_pre(t)
                pt = psum2.tile([80, F], F32, tag="pt")
                for hh_ in range(2):
                    sl = slice(hh_ * 512, (hh_ + 1) * 512)
                    nc.tensor.matmul(pt[:, sl], cblk_bf, et2[:, sl],
                                     start=True, stop=False)
                    nc.tensor.matmul(pt[:, sl], sb["blockmask_bf"], sq[:, sl],
                                     start=False, stop=True)
                dd = p2b.tile([80, F], BF16, tag="dd")
                nc.scalar.activation(out=dd, in_=pt, func=Act.Sqrt,
                                     bias=c2bias[:80, :], scale=1.0)
                hh = p2b.tile([80, F], BF16, tag="hh")
                nc.vector.tensor_scalar(out=hh, in0=dd, scalar1=-DELTA_V,
                                        scalar2=0.0, op0=Alu.add, op1=Alu.max)
                h2 = p2b.tile([80, F], BF16, tag="h2")
                if t % 4 == 3:
                    nc.scalar.square(h2, hh)
                else:
                    nc.vector.tensor_tensor(out=h2, in0=hh, in1=hh,
                                            op=Alu.mult)
                q2 = p2b.tile([80, F], BF16, tag="q2")
                if t % 4 == 1:
                    nc.gpsimd.tensor_mul(q2, h2, mm)
                else:
                    nc.vector.tensor_tensor(out=q2, in0=h2, in1=mm,
                                            op=Alu.mult)
                for hh2 in range(2):
                    sl = slice(hh2 * 512, (hh2 + 1) * 512)
                    nc.tensor.matmul(
                        psI2, sb["kpat_bf"][:80, :], q2[:, sl],
                        start=(t == 0 and hh2 == 0),
                        stop=(t == NT - 1 and hh2 == 1))

            junk5 = singles.tile([K, 512], F32)
            inst5 = singles.tile([K, 1], F32)
            nc.scalar.activation(out=junk5, in_=psI2, func=Act.Copy,
                                 bias=0.0, scale=1.0, accum_out=inst5)
            nc.sync.dma_start(out=o_inst[:].unsqueeze(1), in_=inst5)

    from concourse.library_overlay import lower_extended_insts
    lower_extended_insts(nc)
    _split_multiwait(nc)
    return nc


_NC_CACHE = None


def _get_nc():
    global _NC_CACHE
    if _NC_CACHE is None:
        _NC_CACHE = build_program()
    return _NC_CACHE


def run_device(embedding, maskf, trace=False):
    nc = _get_nc()
    in_maps = [
        {"emb": _to_bf16(np.ascontiguousarray(embedding[b].reshape(D, P))),
         "maskb": _to_bf16(np.ascontiguousarray(maskf[b].reshape(P)))}
        for b in range(B)
    ]
    res = run_bass_kernel_spmd(nc, in_maps, list(range(B)), trace=trace)
    return res


def finalize(per_core):
    """Combine per-image device stats into the 4 reference losses."""
    loss_var_b = np.zeros(B, np.float32)
    loss_dist_b = np.zeros(B, np.float32)
    loss_reg_b = np.zeros(B, np.float32)
    Ns = np.zeros(B, np.float32)
    iu = np.triu(np.ones((K, K), bool), k=1)
    for b in range(B):
        s45 = per_core[b]["o_stats"].astype(np.float32)
        c = per_core[b]["o_c"].astype(np.float32).reshape(K, D)
        inst = per_core[b]["o_inst"].astype(np.float32)
        counts = s45[8::9]
        present = counts > 0
        presentf = present.astype(np.float32)
        N = presentf.sum()
        Ns[b] = N
        inst_mean = inst / np.maximum(counts, 1.0)
        loss_var_b[b] = (inst_mean * presentf).sum() / max(N, 1.0)
        diff = c[:, None, :] - c[None, :, :]
        dist_sq = (diff ** 2).sum(-1)
        pair_mask = present[:, None] & present[None, :] & iu
        safe = np.sqrt(np.where(pair_mask, dist_sq, 1.0))
        term = np.maximum(2.0 * DELTA_D - safe, 0.0) ** 2 * pair_mask
        n_pairs = N * (N - 1.0) / 2.0
        loss_dist_b[b] = term.sum() / (n_pairs if N > 1 else 1.0)
        c_norm = np.sqrt(np.where(present, (c ** 2).sum(-1), 1.0))
        loss_reg_b[b] = (c_norm * presentf).sum() / max(N, 1.0)
    has = (Ns > 0).astype(np.float32)
    denom = max(has.sum(), 1.0)
    loss_var = float((loss_var_b * has).sum() / denom)
    loss_dist = float((loss_dist_b * has).sum() / denom)
    loss_reg = float((loss_reg_b * has).sum() / denom)
    total = ALPHA * loss_var + BETA * loss_dist + GAMMA * loss_reg
    return (np.float32(total), np.float32(loss_var),
            np.float32(loss_dist), np.float32(loss_reg))


def kernel(embedding, instance_mask):
    embedding = np.asarray(embedding, dtype=np.float32)
    maskf = np.asarray(instance_mask).astype(np.float32)
    res = run_device(embedding, maskf, trace=False)
    return finalize(res.results)


# revision 36
# speedup vs baseline: 1.6651x; 1.0358x over previous
"""Discriminative loss kernel for Trainium2 (Bass/Tile), 8-core SPMD.

Data-parallel over batch: core b processes image b (B=8).
Per image, P = 512*1024 pixels, D=8 channels, K=5 labels (0 = background).

pass 1 (segment sums on the PE via pixel-major chunks):
  The flat image [128, 8, 4096] is processed in 4 quarters. Each quarter:
  PE-transpose the 8 e-planes and the label plane into pixel-on-partition
  tiles ([128, 1024] per plane, via 128x128 transposing matmuls through
  PSUM), build one-hot planes ohT[k] with 4x tensor_scalar is_equal, then
  for each 128-pixel chunk accumulate
      psum[k, 0:8] += ohT_chunk^T @ eT_chunk,  psum[k, 8] += sum(ohT_chunk)
  into a single [5, 9] PSUM tile (stationary reloads are pipelined;
  4096 tiny matmuls total). This replaces the DVE masked-product planes,
  the ones-matmul reduction and the ACT accumulate pass of the previous
  version.
tiny device math: centers c = sums/counts, |c_k|^2 bias, and the
  block-diagonal stationary holding -2*c for pass 2 (as before).
pass 2 ((g,d) blocked layout: partition = g*8+d, g=16 pixel groups):
  psum[(g,k),f] = sum_d(-2 c_kd e_d) + |e|^2       (two PE matmuls)
  d = sqrt(psum + C2_k); h = relu(d - 0.5); h2 = h^2
  inst_sum[k] += sum_f h2 * (label==k+1)
  The one-hot arrives via a stride-0 broadcast DMA of the raw labels into
  [80, F] plus a 4x is_equal (no DRAM one-hot round trip).
Host combines the per-image scalars into the final 4 losses.
"""

import os
import sys

import numpy as np

for _p in ("/opt/trn_rl_repo", "/root/.axon_site/_ro/trn_rl_repo"):
    if os.path.isdir(_p) and _p not in sys.path:
        sys.path.insert(0, _p)

import concourse.bass as bass
import concourse.tile as tile
from concourse import mybir
from concourse.bass_utils import run_bass_kernel_spmd

F32 = mybir.dt.float32
BF16 = mybir.dt.bfloat16
F8 = mybir.dt.float8e4
Alu = mybir.AluOpType
Act = mybir.ActivationFunctionType

B, D, H, W = 8, 8, 512, 1024
P = H * W          # 524288 pixels
K = 5
R = 128            # sbuf partitions
COLS = P // R      # 4096 flat cols
NQ = 8             # pass-1 slices
QW = COLS // NQ    # 512 cols per slice
NB = QW // R       # 4 transpose blocks per plane per slice
G = 16             # pass-2 pixel groups
GPP = P // G       # 32768 pixels per group
F = 1024           # pass-2 tile width
NT = GPP // F      # 32 tiles
DELTA_V = 0.5
DELTA_D = 3.0
ALPHA, BETA, GAMMA = 1.0, 1.0, 0.001


def _to_bf16(a):
    import ml_dtypes
    return a.astype(ml_dtypes.bfloat16)


def _to_f8(a):
    import ml_dtypes
    return a.astype(ml_dtypes.float8_e4m3fn)


def _build_consts():
    sel_cnt = np.zeros((R, 40), np.float32)
    sel_sum = np.zeros((R, 40), np.float32)
    for k in range(K):
        for d in range(D):
            sel_cnt[9 * k + 8, 8 * k + d] = 1.0
            sel_sum[9 * k + d, 8 * k + d] = 1.0
    sum5 = np.zeros((R, K), np.float32)
    for k in range(K):
        for d in range(D):
            sum5[8 * k + d, k] = 1.0
    rep80 = np.zeros((R, 80), np.float32)
    for g in range(G):
        for k in range(K):
            rep80[k, 5 * g + k] = 1.0
    smat = np.zeros((R, 80), np.float32)
    for kk in range(K):
        for d in range(D):
            for g in range(G):
                smat[8 * kk + d, 5 * g + kk] = 1.0
    dsel = np.zeros((R, R), np.float32)
    for k in range(K):
        for d in range(D):
            for g in range(G):
                dsel[8 * k + d, 8 * g + d] = 1.0
    blockmask = np.zeros((R, 80), np.float32)
    for g in range(G):
        for d in range(D):
            for k in range(K):
                blockmask[8 * g + d, 5 * g + k] = 1.0
    ones_col = np.ones((R, 1), np.float32)
    kpat = np.zeros((R, K), np.float32)
    kvec = np.zeros((R, 1), np.float32)
    for g in range(G):
        for k in range(K):
            kpat[5 * g + k, k] = 1.0
            kvec[5 * g + k, 0] = float(k + 1)
    scat45 = np.zeros((R, 45), np.float32)
    dmask45 = np.zeros((R, 9), np.float32)
    for k in range(K):
        for dd in range(9):
            scat45[k, 9 * k + dd] = 1.0
            dmask45[9 * k + dd, dd] = 1.0
    f32_pack = dict(sel_cnt=sel_cnt, sel_sum=sel_sum, sum5=sum5,
                    rep80=rep80, smat=smat, dsel=dsel, blockmask=blockmask,
                    ones_col=ones_col, kpat=kpat, kvec=kvec,
                    scat45=scat45, dmask45=dmask45)
    bf_pack = dict(ident_bf=_to_bf16(np.eye(R, dtype=np.float32)),
                   blockmask_bf=_to_bf16(blockmask),
                   kpat_bf=_to_bf16(kpat))
    return f32_pack, bf_pack


def _ap(handle, offset, dims):
    return bass.AP(tensor=handle.tensor if isinstance(handle, bass.AP) else handle,
                   offset=offset, ap=[list(x) for x in dims])


def _tap(t, extra_offset, dims):
    """Sub-AP of an SBUF tile at t.offset + extra_offset."""
    return bass.AP(tensor=t.tensor, offset=t.offset + extra_offset,
                   ap=[list(x) for x in dims])


def _split_multiwait(nc):
    """This container's walrus encodes at most one sync-wait per instruction;
    Tile's tail drain carries one wait per outstanding DMA queue. Hoist the
    extra waits onto single-wait drains inserted just before."""
    n_split = 0
    for blk in nc.m.functions[0].blocks:
        out = []
        changed = False
        for i in blk.instructions:
            si = i.sync_info
            if si is not None and len(si.on_wait) > 1:
                waits = list(si.on_wait)
                for w in waits[:-1]:
                    d = mybir.InstDrain(
                        name=nc.get_next_instruction_name(), ins=[], outs=[])
                    d.engine = i.engine
                    d.sync_info = mybir.SyncInfo(on_wait=[w], on_update=[])
                    out.append(d)
                    n_split += 1
                i.sync_info = mybir.SyncInfo(
                    on_wait=[waits[-1]], on_update=list(si.on_update))
                changed = True
            out.append(i)
        if changed:
            blk.instructions = out
    return n_split


def build_program():
    nc = bass.Bass()
    emb = nc.declare_dram_parameter("emb", [D, P], BF16, isOutput=False)
    maskb = nc.declare_dram_parameter("maskb", [P], BF16, isOutput=False)
    o_stats = nc.declare_dram_parameter("o_stats", [45], F32, isOutput=True)
    o_c = nc.declare_dram_parameter("o_c", [40], F32, isOutput=True)
    o_inst = nc.declare_dram_parameter("o_inst", [K], F32, isOutput=True)

    f32_pack, bf_pack = _build_consts()
    cat_f32 = nc.inline_tensor(
        np.concatenate([v for v in f32_pack.values()], axis=1), name="c_f32")
    cat_bf = nc.inline_tensor(
        np.concatenate([v for v in bf_pack.values()], axis=1), name="c_bf")

    with tile.TileContext(nc) as tc:
        with tc.tile_pool(name="singles", bufs=1) as singles, \
             tc.tile_pool(name="ld", bufs=4) as ldp, \
             tc.tile_pool(name="tr", bufs=3) as trp, \
             tc.tile_pool(name="p2a", bufs=14) as p2a, \
             tc.tile_pool(name="p2l", bufs=6) as p2l, \
             tc.tile_pool(name="p2b", bufs=3) as p2b, \
             tc.tile_pool(name="psumA", bufs=3, space="PSUM") as psumA, \
             tc.tile_pool(name="psumB", bufs=1, space="PSUM") as psumB, \
             tc.tile_pool(name="psum2", bufs=2, space="PSUM") as psum2:

            # first data slice BEFORE the constants: the dozen constant
            # DMAs otherwise serialize ~13us of dead time at the head.
            ebf0 = ldp.tile([R, D, QW], BF16, tag="ebf", name="ebf0")
            lbq0 = ldp.tile([R, QW], BF16, tag="lbq", name="lb0")
            nc.sync.dma_start(
                out=ebf0, in_=_ap(emb, 0, [[COLS, R], [P, D], [1, QW]]))
            nc.sync.dma_start(
                out=lbq0, in_=_ap(maskb, 0, [[COLS, R], [1, QW]]))

            # load constants: two packed DMAs (bf16 first: transposes
            # need the identity), then carve slice views
            sb = {}
            wbf = sum(v.shape[1] for v in bf_pack.values())
            wf32 = sum(v.shape[1] for v in f32_pack.values())
            tbf = singles.tile([R, wbf], BF16, tag="c_bf")
            nc.sync.dma_start(out=tbf, in_=cat_bf[:])
            tf32 = singles.tile([R, wf32], F32, tag="c_f32")
            nc.sync.dma_start(out=tf32, in_=cat_f32[:])
            off = 0
            for name, v in bf_pack.items():
                sb[name] = tbf[:, off:off + v.shape[1]]
                off += v.shape[1]
            off = 0
            for name, v in f32_pack.items():
                sb[name] = tf32[:, off:off + v.shape[1]]
                off += v.shape[1]

            # constants used as activation biases
            for cval in (0.0, -DELTA_V):
                ct = singles.tile([R, 1], F32, tag=f"bias_{cval}")
                nc.vector.memset(ct, cval)
                nc.const_aps.aps[(F32, cval)] = ct[:]

            # ---------------- pass 1 ----------------
            # per quarter: transpose 8 e-planes + label plane to pixel-major,
            # one-hot via 4x is_equal, then 1024 accumulating chunk matmuls.
            ps59 = psumB.tile([K, D + 1], F32, tag="small")

            def p2_pre(t):
                """Center-independent pass-2 tile work (DMA, one-hot, |e|^2).
                Emitted inside pass 1 for the first NQ tiles to fill engine
                slack and shorten the pass-1 -> pass-2 transition."""
                et2 = p2a.tile([R, F], BF16, tag="et2", name=f"et2_{t}")
                nc.sync.dma_start(
                    out=et2, in_=_ap(emb, t * F, [[GPP, G], [P, D], [1, F]]))
                lb2 = p2l.tile([80, F], BF16, tag="lb2", name=f"lb2_{t}")
                nc.sync.dma_start(
                    out=lb2, in_=_ap(maskb, t * F, [[GPP, G], [0, K], [1, F]]))
                mm = p2a.tile([80, F], BF16, tag="mm", name=f"mm_{t}")
                nc.vector.tensor_scalar(
                    out=mm, in0=lb2, scalar1=sb["kvec"][:80, :],
                    scalar2=None, op0=Alu.is_equal)
                sq = p2a.tile([R, F], BF16, tag="sq", name=f"sq_{t}")
                if t % 8 in (0, 4):
                    nc.scalar.square(sq, et2)
                elif t % 8 in (2, 6):
                    nc.vector.tensor_tensor(out=sq, in0=et2, in1=et2,
                                            op=Alu.mult)
                else:
                    nc.gpsimd.tensor_mul(sq, et2, et2)
                return et2, mm, sq

            stage = {}
            pre2 = {}
            for q in range(NQ + 1):
                if q < NQ:
                    base = q * QW
                    if q == 0:
                        ebf, lbq = ebf0, lbq0
                    else:
                        ebf = ldp.tile([R, D, QW], BF16, tag="ebf",
                                       name=f"ebf{q}")
                        lbq = ldp.tile([R, QW], BF16, tag="lbq",
                                       name=f"lb{q}")
                        nc.sync.dma_start(
                            out=ebf,
                            in_=_ap(emb, base, [[COLS, R], [P, D], [1, QW]]))
                        nc.sync.dma_start(
                            out=lbq,
                            in_=_ap(maskb, base, [[COLS, R], [1, QW]]))

                    eT = trp.tile([R, D + 1, QW], BF16, tag="eT",
                                  name=f"eT{q}")
                    ohT = trp.tile([R, K, QW], BF16, tag="ohT",
                                   name=f"ohT{q}")
                    lT = trp.tile([R, QW], BF16, tag="lT", name=f"lT{q}")
                    if q < 3:
                        nc.gpsimd.memset(eT[:, D, :], 1.0)

                    # labels first (one-hots start while e transposes run)
                    pst = psumA.tile([R, QW], BF16, tag="trans")
                    for b in range(NB):
                        nc.tensor.transpose(
                            pst[:, b * R:(b + 1) * R],
                            lbq[:, b * R:(b + 1) * R], sb["ident_bf"])
                    nc.scalar.copy(out=lT, in_=pst)
                    for k in range(K):
                        nc.vector.tensor_scalar(
                            out=ohT[:, k, :], in0=lT, scalar1=float(k + 1),
                            scalar2=None, op0=Alu.is_equal)

                    for d in range(D):
                        pse = psumA.tile([R, QW], BF16, tag="trans",
                                         name=f"pse{q}_{d}")
                        for b in range(NB):
                            nc.tensor.transpose(
                                pse[:, b * R:(b + 1) * R],
                                ebf[:, d, b * R:(b + 1) * R], sb["ident_bf"])
                        # GPSIMD has no PSUM access: ACT/DVE only here
                        if d % 2 == 1:
                            nc.vector.tensor_copy(out=eT[:, d, :], in_=pse)
                        else:
                            nc.scalar.copy(out=eT[:, d, :], in_=pse)
                    stage[q] = (eT, ohT)
                    pre2[q] = p2_pre(q)

                if q >= 1:
                    # chunk matmuls for the PREVIOUS slice: PE runs these
                    # while this slice's copies drain on ACT/DVE/Pool.
                    eTp, ohTp = stage.pop(q - 1)
                    for c in range(QW):
                        nc.tensor.matmul(
                            ps59,
                            _tap(ohTp, c, [[ohTp.ap[0][0], R], [QW, K]]),
                            _tap(eTp, c, [[eTp.ap[0][0], R], [QW, D + 1]]),
                            start=(q == 1 and c == 0),
                            stop=(q == NQ and c == QW - 1))

            for t_ in range(NQ, NQ + 6):
                pre2[t_] = p2_pre(t_)

            # [5, 9] psum -> 45-long stats vector, fully on-chip:
            # scatter matmul to [45, 9], diagonal mask, free-dim reduce.
            # (a DMA rearrange here costs ~3us of all-engine stall)
            s59 = singles.tile([K, D + 1], F32)
            nc.scalar.copy(out=s59, in_=ps59)
            sb45 = singles.tile([R, 1], F32)
            nc.vector.memset(sb45, 0.0)
            ps45x9 = psumB.tile([45, 9], F32, tag="small")
            nc.tensor.matmul(ps45x9, sb["scat45"][:K, :], s59,
                             start=True, stop=True)
            s45x9 = singles.tile([45, 9], F32)
            nc.vector.tensor_tensor(out=s45x9, in0=ps45x9,
                                    in1=sb["dmask45"][:45, :], op=Alu.mult)
            nc.vector.tensor_reduce(
                out=sb45[:45, :], in_=s45x9,
                axis=mybir.AxisListType.X, op=Alu.add)
            nc.sync.dma_start(out=o_stats[:].unsqueeze(1), in_=sb45[:45, :])

            # ---------------- tiny math: centers ----------------
            ps40a = psumB.tile([40, 1], F32, tag="small")
            nc.tensor.matmul(ps40a, sb["sel_cnt"], sb45, start=True, stop=True)
            ps40b = psumB.tile([40, 1], F32, tag="small")
            nc.tensor.matmul(ps40b, sb["sel_sum"], sb45, start=True, stop=True)
            cntc = singles.tile([R, 1], F32)
            nc.vector.memset(cntc, 0.0)
            nc.vector.tensor_scalar(out=cntc[:40, :], in0=ps40a, scalar1=1.0,
                                    scalar2=None, op0=Alu.max)
            inv = singles.tile([R, 1], F32)
            nc.vector.memset(inv, 0.0)
            nc.vector.reciprocal(out=inv[:40, :], in_=cntc[:40, :])
            c40 = singles.tile([R, 1], F32)
            nc.vector.memset(c40, 0.0)
            nc.vector.tensor_tensor(out=c40[:40, :], in0=ps40b, in1=inv[:40, :],
                                    op=Alu.mult)
            nc.sync.dma_start(out=o_c[:].unsqueeze(1), in_=c40[:40, :])
            cm2 = singles.tile([R, 1], F32)
            nc.vector.memset(cm2, 0.0)
            nc.vector.tensor_scalar(out=cm2[:40, :], in0=c40[:40, :],
                                    scalar1=-2.0, scalar2=None, op0=Alu.mult)
            csq = singles.tile([R, 1], F32)
            nc.vector.memset(csq, 0.0)
            nc.vector.tensor_tensor(out=csq[:40, :], in0=c40[:40, :],
                                    in1=c40[:40, :], op=Alu.mult)
            ps5 = psumB.tile([K, 1], F32, tag="small")
            nc.tensor.matmul(ps5, sb["sum5"], csq, start=True, stop=True)
            c2sb = singles.tile([R, 1], F32)
            nc.vector.memset(c2sb, 0.0)
            nc.scalar.copy(out=c2sb[:K, :], in_=ps5)
            ps80 = psumB.tile([80, 1], F32, tag="small")
            nc.tensor.matmul(ps80, sb["rep80"], c2sb, start=True, stop=True)
            c2bias = singles.tile([R, 1], F32)
            nc.vector.memset(c2bias, 0.0)
            nc.scalar.copy(out=c2bias[:80, :], in_=ps80)

            # block-diagonal stationary: cblk[8g+d, 5g+k] = -2*c[k,d]
            rhsS = singles.tile([R, 80], F32)
            nc.vector.tensor_scalar(out=rhsS, in0=sb["smat"], scalar1=cm2,
                                    scalar2=None, op0=Alu.mult)
            psD = psumB.tile([R, 80], F32, tag="small")
            nc.tensor.matmul(psD, sb["dsel"], rhsS, start=True, stop=True)
            cblk = singles.tile([R, 80], F32)
            nc.vector.tensor_tensor(out=cblk, in0=psD, in1=sb["blockmask"],
                                    op=Alu.mult)
            cblk_bf = singles.tile([R, 80], BF16)
            nc.vector.tensor_scalar(out=cblk_bf, in0=cblk, scalar1=1.0,
                                    scalar2=None, op0=Alu.mult)

            # ---------------- pass 2 ----------------
            psI2 = psumB.tile([K, 512], F32, tag="small")
            for t in range(NT):
                if t in pre2:
                    et2, mm, sq = pre2.pop(t)
                else:
                    et2, mm, sq = p2_pre(t)
                pt = psum2.tile([80, F], F32, tag="pt")
                for hh_ in range(2):
                    sl = slice(hh_ * 512, (hh_ + 1) * 512)
                    nc.tensor.matmul(pt[:, sl], cblk_bf, et2[:, sl],
                                     start=True, stop=False)
                    nc.tensor.matmul(pt[:, sl], sb["blockmask_bf"], sq[:, sl],
                                     start=False, stop=True)
                dd = p2b.tile([80, F], BF16, tag="dd")
                nc.scalar.activation(out=dd, in_=pt, func=Act.Sqrt,
                                     bias=c2bias[:80, :], scale=1.0)
                hh = p2b.tile([80, F], BF16, tag="hh")
                nc.vector.tensor_scalar(out=hh, in0=dd, scalar1=-DELTA_V,
                                        scalar2=0.0, op0=Alu.add, op1=Alu.max)
                h2 = p2b.tile([80, F], BF16, tag="h2")
                if t % 4 == 3:
                    nc.scalar.square(h2, hh)
                else:
                    nc.vector.tensor_tensor(out=h2, in0=hh, in1=hh,
                                            op=Alu.mult)
                q2 = p2b.tile([80, F], BF16, tag="q2")
                if t % 4 == 1:
                    nc.gpsimd.tensor_mul(q2, h2, mm)
                else:
                    nc.vector.tensor_tensor(out=q2, in0=h2, in1=mm,
                                            op=Alu.mult)
                for hh2 in range(2):
                    sl = slice(hh2 * 512, (hh2 + 1) * 512)
                    nc.tensor.matmul(
                        psI2, sb["kpat_bf"][:80, :], q2[:, sl],
                        start=(t == 0 and hh2 == 0),
                        stop=(t == NT - 1 and hh2 == 1))

            junk5 = singles.tile([K, 512], F32)
            inst5 = singles.tile([K, 1], F32)
            nc.scalar.activation(out=junk5, in_=psI2, func=Act.Copy,
                                 bias=0.0, scale=1.0, accum_out=inst5)
            nc.sync.dma_start(out=o_inst[:].unsqueeze(1), in_=inst5)

    from concourse.library_overlay import lower_extended_insts
    lower_extended_insts(nc)
    _split_multiwait(nc)
    return nc


_NC_CACHE = None


def _get_nc():
    global _NC_CACHE
    if _NC_CACHE is None:
        _NC_CACHE = build_program()
    return _NC_CACHE


def run_device(embedding, maskf, trace=False):
    nc = _get_nc()
    in_maps = [
        {"emb": _to_bf16(np.ascontiguousarray(embedding[b].reshape(D, P))),
         "maskb": _to_bf16(np.ascontiguousarray(maskf[b].reshape(P)))}
        for b in range(B)
    ]
    res = run_bass_kernel_spmd(nc, in_maps, list(range(B)), trace=trace)
    return res


def finalize(per_core):
    """Combine per-image device stats into the 4 reference losses."""
    loss_var_b = np.zeros(B, np.float32)
    loss_dist_b = np.zeros(B, np.float32)
    loss_reg_b = np.zeros(B, np.float32)
    Ns = np.zeros(B, np.float32)
    iu = np.triu(np.ones((K, K), bool), k=1)
    for b in range(B):
        s45 = per_core[b]["o_stats"].astype(np.float32)
        c = per_core[b]["o_c"].astype(np.float32).reshape(K, D)
        inst = per_core[b]["o_inst"].astype(np.float32)
        counts = s45[8::9]
        present = counts > 0
        presentf = present.astype(np.float32)
        N = presentf.sum()
        Ns[b] = N
        inst_mean = inst / np.maximum(counts, 1.0)
        loss_var_b[b] = (inst_mean * presentf).sum() / max(N, 1.0)
        diff = c[:, None, :] - c[None, :, :]
        dist_sq = (diff ** 2).sum(-1)
        pair_mask = present[:, None] & present[None, :] & iu
        safe = np.sqrt(np.where(pair_mask, dist_sq, 1.0))
        term = np.maximum(2.0 * DELTA_D - safe, 0.0) ** 2 * pair_mask
        n_pairs = N * (N - 1.0) / 2.0
        loss_dist_b[b] = term.sum() / (n_pairs if N > 1 else 1.0)
        c_norm = np.sqrt(np.where(present, (c ** 2).sum(-1), 1.0))
        loss_reg_b[b] = (c_norm * presentf).sum() / max(N, 1.0)
    has = (Ns > 0).astype(np.float32)
    denom = max(has.sum(), 1.0)
    loss_var = float((loss_var_b * has).sum() / denom)
    loss_dist = float((loss_dist_b * has).sum() / denom)
    loss_reg = float((loss_reg_b * has).sum() / denom)
    total = ALPHA * loss_var + BETA * loss_dist + GAMMA * loss_reg
    return (np.float32(total), np.float32(loss_var),
            np.float32(loss_dist), np.float32(loss_reg))


def kernel(embedding, instance_mask):
    embedding = np.asarray(embedding, dtype=np.float32)
    maskf = np.asarray(instance_mask).astype(np.float32)
    res = run_device(embedding, maskf, trace=False)
    return finalize(res.results)


# revision 45
# speedup vs baseline: 1.6871x; 1.0132x over previous
"""Discriminative loss kernel for Trainium2 (Bass/Tile), 8-core SPMD.

Data-parallel over batch: core b processes image b (B=8).
Per image, P = 512*1024 pixels, D=8 channels, K=5 labels (0 = background).

pass 1 (segment sums on the PE via pixel-major chunks):
  The flat image [128, 8, 4096] is processed in 4 quarters. Each quarter:
  PE-transpose the 8 e-planes and the label plane into pixel-on-partition
  tiles ([128, 1024] per plane, via 128x128 transposing matmuls through
  PSUM), build one-hot planes ohT[k] with 4x tensor_scalar is_equal, then
  for each 128-pixel chunk accumulate
      psum[k, 0:8] += ohT_chunk^T @ eT_chunk,  psum[k, 8] += sum(ohT_chunk)
  into a single [5, 9] PSUM tile (stationary reloads are pipelined;
  4096 tiny matmuls total). This replaces the DVE masked-product planes,
  the ones-matmul reduction and the ACT accumulate pass of the previous
  version.
tiny device math: centers c = sums/counts, |c_k|^2 bias, and the
  block-diagonal stationary holding -2*c for pass 2 (as before).
pass 2 ((g,d) blocked layout: partition = g*8+d, g=16 pixel groups):
  psum[(g,k),f] = sum_d(-2 c_kd e_d) + |e|^2       (two PE matmuls)
  d = sqrt(psum + C2_k); h = relu(d - 0.5); h2 = h^2
  inst_sum[k] += sum_f h2 * (label==k+1)
  The one-hot arrives via a stride-0 broadcast DMA of the raw labels into
  [80, F] plus a 4x is_equal (no DRAM one-hot round trip).
Host combines the per-image scalars into the final 4 losses.
"""

import os
import sys

import numpy as np

for _p in ("/opt/trn_rl_repo", "/root/.axon_site/_ro/trn_rl_repo"):
    if os.path.isdir(_p) and _p not in sys.path:
        sys.path.insert(0, _p)

import concourse.bass as bass
import concourse.tile as tile
from concourse import mybir
from concourse.bass_utils import run_bass_kernel_spmd

F32 = mybir.dt.float32
BF16 = mybir.dt.bfloat16
F8 = mybir.dt.float8e4
Alu = mybir.AluOpType
Act = mybir.ActivationFunctionType

B, D, H, W = 8, 8, 512, 1024
P = H * W          # 524288 pixels
K = 5
R = 128            # sbuf partitions
COLS = P // R      # 4096 flat cols
NQ = 8             # pass-1 slices
QW = COLS // NQ    # 512 cols per slice
NB = QW // R       # 4 transpose blocks per plane per slice
G = 16             # pass-2 pixel groups
GPP = P // G       # 32768 pixels per group
F = 1024           # pass-2 tile width
NT = GPP // F      # 32 tiles
DELTA_V = 0.5
DELTA_D = 3.0
ALPHA, BETA, GAMMA = 1.0, 1.0, 0.001


def _to_bf16(a):
    import ml_dtypes
    return a.astype(ml_dtypes.bfloat16)


def _to_f8(a):
    import ml_dtypes
    return a.astype(ml_dtypes.float8_e4m3fn)


def _build_consts():
    sel_cnt = np.zeros((R, 40), np.float32)
    sel_sum = np.zeros((R, 40), np.float32)
    for k in range(K):
        for d in range(D):
            sel_cnt[9 * k + 8, 8 * k + d] = 1.0
            sel_sum[9 * k + d, 8 * k + d] = 1.0
    sum5 = np.zeros((R, K), np.float32)
    for k in range(K):
        for d in range(D):
            sum5[8 * k + d, k] = 1.0
    rep80 = np.zeros((R, 80), np.float32)
    for g in range(G):
        for k in range(K):
            rep80[k, 5 * g + k] = 1.0
    smat = np.zeros((R, 80), np.float32)
    for kk in range(K):
        for d in range(D):
            for g in range(G):
                smat[8 * kk + d, 5 * g + kk] = 1.0
    dsel = np.zeros((R, R), np.float32)
    for k in range(K):
        for d in range(D):
            for g in range(G):
                dsel[8 * k + d, 8 * g + d] = 1.0
    blockmask = np.zeros((R, 80), np.float32)
    for g in range(G):
        for d in range(D):
            for k in range(K):
                blockmask[8 * g + d, 5 * g + k] = 1.0
    ones_col = np.ones((R, 1), np.float32)
    kpat = np.zeros((R, K), np.float32)
    kvec = np.zeros((R, 1), np.float32)
    for g in range(G):
        for k in range(K):
            kpat[5 * g + k, k] = 1.0
            kvec[5 * g + k, 0] = float(k + 1)
    scat45 = np.zeros((R, 45), np.float32)
    dmask45 = np.zeros((R, 9), np.float32)
    for k in range(K):
        for dd in range(9):
            scat45[k, 9 * k + dd] = 1.0
            dmask45[9 * k + dd, dd] = 1.0
    f32_pack = dict(sel_cnt=sel_cnt, sel_sum=sel_sum, sum5=sum5,
                    rep80=rep80, smat=smat, dsel=dsel, blockmask=blockmask,
                    ones_col=ones_col, kpat=kpat, kvec=kvec,
                    scat45=scat45, dmask45=dmask45)
    bf_pack = dict(ident_bf=_to_bf16(np.eye(R, dtype=np.float32)),
                   blockmask_bf=_to_bf16(blockmask),
                   kpat_bf=_to_bf16(kpat))
    return f32_pack, bf_pack


def _ap(handle, offset, dims):
    return bass.AP(tensor=handle.tensor if isinstance(handle, bass.AP) else handle,
                   offset=offset, ap=[list(x) for x in dims])


def _tap(t, extra_offset, dims):
    """Sub-AP of an SBUF tile at t.offset + extra_offset."""
    return bass.AP(tensor=t.tensor, offset=t.offset + extra_offset,
                   ap=[list(x) for x in dims])


def _split_multiwait(nc):
    """This container's walrus encodes at most one sync-wait per instruction;
    Tile's tail drain carries one wait per outstanding DMA queue. Hoist the
    extra waits onto single-wait drains inserted just before."""
    n_split = 0
    for blk in nc.m.functions[0].blocks:
        out = []
        changed = False
        for i in blk.instructions:
            si = i.sync_info
            if si is not None and len(si.on_wait) > 1:
                waits = list(si.on_wait)
                for w in waits[:-1]:
                    d = mybir.InstDrain(
                        name=nc.get_next_instruction_name(), ins=[], outs=[])
                    d.engine = i.engine
                    d.sync_info = mybir.SyncInfo(on_wait=[w], on_update=[])
                    out.append(d)
                    n_split += 1
                i.sync_info = mybir.SyncInfo(
                    on_wait=[waits[-1]], on_update=list(si.on_update))
                changed = True
            out.append(i)
        if changed:
            blk.instructions = out
    return n_split


def build_program():
    nc = bass.Bass()
    emb = nc.declare_dram_parameter("emb", [D, P], BF16, isOutput=False)
    maskb = nc.declare_dram_parameter("maskb", [P], BF16, isOutput=False)
    o_stats = nc.declare_dram_parameter("o_stats", [45], F32, isOutput=True)
    o_c = nc.declare_dram_parameter("o_c", [40], F32, isOutput=True)
    o_inst = nc.declare_dram_parameter("o_inst", [K], F32, isOutput=True)

    f32_pack, bf_pack = _build_consts()
    cat_f32 = nc.inline_tensor(
        np.concatenate([v for v in f32_pack.values()], axis=1), name="c_f32")
    cat_bf = nc.inline_tensor(
        np.concatenate([v for v in bf_pack.values()], axis=1), name="c_bf")

    with tile.TileContext(nc) as tc:
        with tc.tile_pool(name="singles", bufs=1) as singles, \
             tc.tile_pool(name="ld", bufs=3) as ldp, \
             tc.tile_pool(name="tr", bufs=3) as trp, \
             tc.tile_pool(name="p2a", bufs=16) as p2a, \
             tc.tile_pool(name="p2l", bufs=4) as p2l, \
             tc.tile_pool(name="p2b", bufs=3) as p2b, \
             tc.tile_pool(name="psumA", bufs=3, space="PSUM") as psumA, \
             tc.tile_pool(name="psumB", bufs=1, space="PSUM") as psumB, \
             tc.tile_pool(name="psum2", bufs=2, space="PSUM") as psum2:

            # first data slice BEFORE the constants: the dozen constant
            # DMAs otherwise serialize ~13us of dead time at the head.
            ebf0 = ldp.tile([R, D, QW], BF16, tag="ebf", name="ebf0")
            lbq0 = ldp.tile([R, QW], BF16, tag="lbq", name="lb0")
            nc.sync.dma_start(
                out=ebf0, in_=_ap(emb, 0, [[COLS, R], [P, D], [1, QW]]))
            nc.sync.dma_start(
                out=lbq0, in_=_ap(maskb, 0, [[COLS, R], [1, QW]]))

            # load constants: two packed DMAs (bf16 first: transposes
            # need the identity), then carve slice views
            sb = {}
            wbf = sum(v.shape[1] for v in bf_pack.values())
            wf32 = sum(v.shape[1] for v in f32_pack.values())
            tbf = singles.tile([R, wbf], BF16, tag="c_bf")
            nc.sync.dma_start(out=tbf, in_=cat_bf[:])
            tf32 = singles.tile([R, wf32], F32, tag="c_f32")
            nc.sync.dma_start(out=tf32, in_=cat_f32[:])
            off = 0
            for name, v in bf_pack.items():
                sb[name] = tbf[:, off:off + v.shape[1]]
                off += v.shape[1]
            off = 0
            for name, v in f32_pack.items():
                sb[name] = tf32[:, off:off + v.shape[1]]
                off += v.shape[1]

            # constants used as activation biases
            for cval in (0.0, -DELTA_V):
                ct = singles.tile([R, 1], F32, tag=f"bias_{cval}")
                nc.vector.memset(ct, cval)
                nc.const_aps.aps[(F32, cval)] = ct[:]

            # ---------------- pass 1 ----------------
            # per quarter: transpose 8 e-planes + label plane to pixel-major,
            # one-hot via 4x is_equal, then 1024 accumulating chunk matmuls.
            ps59 = psumB.tile([K, D + 1], F32, tag="small")

            def p2_pre(t):
                """Center-independent pass-2 tile work (DMA, one-hot, |e|^2).
                Emitted inside pass 1 for the first NQ tiles to fill engine
                slack and shorten the pass-1 -> pass-2 transition."""
                et2 = p2a.tile([R, F], BF16, tag="et2", name=f"et2_{t}")
                nc.sync.dma_start(
                    out=et2, in_=_ap(emb, t * F, [[GPP, G], [P, D], [1, F]]))
                lb2 = p2l.tile([80, F], BF16, tag="lb2", name=f"lb2_{t}")
                nc.sync.dma_start(
                    out=lb2, in_=_ap(maskb, t * F, [[GPP, G], [0, K], [1, F]]))
                mm = p2a.tile([80, F], BF16, tag="mm", name=f"mm_{t}")
                nc.vector.tensor_scalar(
                    out=mm, in0=lb2, scalar1=sb["kvec"][:80, :],
                    scalar2=None, op0=Alu.is_equal)
                sq = p2a.tile([R, F], BF16, tag="sq", name=f"sq_{t}")
                if t % 8 in (0, 4):
                    nc.scalar.square(sq, et2)
                elif t % 8 in (2, 6):
                    nc.vector.tensor_tensor(out=sq, in0=et2, in1=et2,
                                            op=Alu.mult)
                else:
                    nc.gpsimd.tensor_mul(sq, et2, et2)
                return et2, mm, sq

            stage = {}
            pre2 = {}
            for q in range(NQ + 1):
                if q < NQ:
                    base = q * QW
                    if q == 0:
                        ebf, lbq = ebf0, lbq0
                    else:
                        ebf = ldp.tile([R, D, QW], BF16, tag="ebf",
                                       name=f"ebf{q}")
                        lbq = ldp.tile([R, QW], BF16, tag="lbq",
                                       name=f"lb{q}")
                        nc.sync.dma_start(
                            out=ebf,
                            in_=_ap(emb, base, [[COLS, R], [P, D], [1, QW]]))
                        nc.sync.dma_start(
                            out=lbq,
                            in_=_ap(maskb, base, [[COLS, R], [1, QW]]))

                    eT = trp.tile([R, D + 1, QW], BF16, tag="eT",
                                  name=f"eT{q}")
                    ohT = trp.tile([R, K, QW], BF16, tag="ohT",
                                   name=f"ohT{q}")
                    lT = trp.tile([R, QW], BF16, tag="lT", name=f"lT{q}")
                    if q < 3:
                        nc.gpsimd.memset(eT[:, D, :], 1.0)

                    # labels first (one-hots start while e transposes run)
                    pst = psumA.tile([R, QW], BF16, tag="trans")
                    for b in range(NB):
                        nc.tensor.transpose(
                            pst[:, b * R:(b + 1) * R],
                            lbq[:, b * R:(b + 1) * R], sb["ident_bf"])
                    nc.scalar.copy(out=lT, in_=pst)
                    for k in range(K):
                        nc.vector.tensor_scalar(
                            out=ohT[:, k, :], in0=lT, scalar1=float(k + 1),
                            scalar2=None, op0=Alu.is_equal)

                    for d in range(D):
                        pse = psumA.tile([R, QW], BF16, tag="trans",
                                         name=f"pse{q}_{d}")
                        for b in range(NB):
                            nc.tensor.transpose(
                                pse[:, b * R:(b + 1) * R],
                                ebf[:, d, b * R:(b + 1) * R], sb["ident_bf"])
                        # GPSIMD has no PSUM access: ACT/DVE only here
                        if d % 2 == 1:
                            nc.vector.tensor_copy(out=eT[:, d, :], in_=pse)
                        else:
                            nc.scalar.copy(out=eT[:, d, :], in_=pse)
                    stage[q] = (eT, ohT)
                    pre2[q] = p2_pre(q)

                if q >= 1:
                    # chunk matmuls for the PREVIOUS slice: PE runs these
                    # while this slice's copies drain on ACT/DVE/Pool.
                    eTp, ohTp = stage.pop(q - 1)
                    for c in range(QW):
                        nc.tensor.matmul(
                            ps59,
                            _tap(ohTp, c, [[ohTp.ap[0][0], R], [QW, K]]),
                            _tap(eTp, c, [[eTp.ap[0][0], R], [QW, D + 1]]),
                            start=(q == 1 and c == 0),
                            stop=(q == NQ and c == QW - 1))

            for t_ in range(NQ, NQ + 8):
                pre2[t_] = p2_pre(t_)

            # [5, 9] psum -> 45-long stats vector, fully on-chip:
            # scatter matmul to [45, 9], diagonal mask, free-dim reduce.
            # (a DMA rearrange here costs ~3us of all-engine stall)
            s59 = singles.tile([K, D + 1], F32)
            nc.scalar.copy(out=s59, in_=ps59)
            sb45 = singles.tile([R, 1], F32)
            nc.vector.memset(sb45, 0.0)
            ps45x9 = psumB.tile([45, 9], F32, tag="small")
            nc.tensor.matmul(ps45x9, sb["scat45"][:K, :], s59,
                             start=True, stop=True)
            s45x9 = singles.tile([45, 9], F32)
            nc.vector.tensor_tensor(out=s45x9, in0=ps45x9,
                                    in1=sb["dmask45"][:45, :], op=Alu.mult)
            nc.vector.tensor_reduce(
                out=sb45[:45, :], in_=s45x9,
                axis=mybir.AxisListType.X, op=Alu.add)
            nc.sync.dma_start(out=o_stats[:].unsqueeze(1), in_=sb45[:45, :])

            # ---------------- tiny math: centers ----------------
            ps40a = psumB.tile([40, 1], F32, tag="small")
            nc.tensor.matmul(ps40a, sb["sel_cnt"], sb45, start=True, stop=True)
            ps40b = psumB.tile([40, 1], F32, tag="small")
            nc.tensor.matmul(ps40b, sb["sel_sum"], sb45, start=True, stop=True)
            cntc = singles.tile([R, 1], F32)
            nc.vector.memset(cntc, 0.0)
            nc.vector.tensor_scalar(out=cntc[:40, :], in0=ps40a, scalar1=1.0,
                                    scalar2=None, op0=Alu.max)
            inv = singles.tile([R, 1], F32)
            nc.vector.memset(inv, 0.0)
            nc.vector.reciprocal(out=inv[:40, :], in_=cntc[:40, :])
            c40 = singles.tile([R, 1], F32)
            nc.vector.memset(c40, 0.0)
            nc.vector.tensor_tensor(out=c40[:40, :], in0=ps40b, in1=inv[:40, :],
                                    op=Alu.mult)
            nc.sync.dma_start(out=o_c[:].unsqueeze(1), in_=c40[:40, :])
            cm2 = singles.tile([R, 1], F32)
            nc.vector.memset(cm2, 0.0)
            nc.vector.tensor_scalar(out=cm2[:40, :], in0=c40[:40, :],
                                    scalar1=-2.0, scalar2=None, op0=Alu.mult)
            csq = singles.tile([R, 1], F32)
            nc.vector.memset(csq, 0.0)
            nc.vector.tensor_tensor(out=csq[:40, :], in0=c40[:40, :],
                                    in1=c40[:40, :], op=Alu.mult)
            ps5 = psumB.tile([K, 1], F32, tag="small")
            nc.tensor.matmul(ps5, sb["sum5"], csq, start=True, stop=True)
            c2sb = singles.tile([R, 1], F32)
            nc.vector.memset(c2sb, 0.0)
            nc.scalar.copy(out=c2sb[:K, :], in_=ps5)
            ps80 = psumB.tile([80, 1], F32, tag="small")
            nc.tensor.matmul(ps80, sb["rep80"], c2sb, start=True, stop=True)
            c2bias = singles.tile([R, 1], F32)
            nc.vector.memset(c2bias, 0.0)
            nc.scalar.copy(out=c2bias[:80, :], in_=ps80)

            # block-diagonal stationary: cblk[8g+d, 5g+k] = -2*c[k,d]
            rhsS = singles.tile([R, 80], F32)
            nc.vector.tensor_scalar(out=rhsS, in0=sb["smat"], scalar1=cm2,
                                    scalar2=None, op0=Alu.mult)
            psD = psumB.tile([R, 80], F32, tag="small")
            nc.tensor.matmul(psD, sb["dsel"], rhsS, start=True, stop=True)
            cblk = singles.tile([R, 80], F32)
            nc.vector.tensor_tensor(out=cblk, in0=psD, in1=sb["blockmask"],
                                    op=Alu.mult)
            cblk_bf = singles.tile([R, 80], BF16)
            nc.vector.tensor_scalar(out=cblk_bf, in0=cblk, scalar1=1.0,
                                    scalar2=None, op0=Alu.mult)

            # ---------------- pass 2 ----------------
            psI2 = psumB.tile([K, 512], F32, tag="small")
            for t in range(NT):
                if t in pre2:
                    et2, mm, sq = pre2.pop(t)
                else:
                    et2, mm, sq = p2_pre(t)
                pt = psum2.tile([80, F], F32, tag="pt")
                for hh_ in range(2):
                    sl = slice(hh_ * 512, (hh_ + 1) * 512)
                    nc.tensor.matmul(pt[:, sl], cblk_bf, et2[:, sl],
                                     start=True, stop=False)
                    nc.tensor.matmul(pt[:, sl], sb["blockmask_bf"], sq[:, sl],
                                     start=False, stop=True)
                dd = p2b.tile([80, F], BF16, tag="dd")
                nc.scalar.activation(out=dd, in_=pt, func=Act.Sqrt,
                                     bias=c2bias[:80, :], scale=1.0)
                hh = p2b.tile([80, F], BF16, tag="hh")
                nc.vector.tensor_scalar(out=hh, in0=dd, scalar1=-DELTA_V,
                                        scalar2=0.0, op0=Alu.add, op1=Alu.max)
                h2 = p2b.tile([80, F], BF16, tag="h2")
                if t % 4 == 3:
                    nc.scalar.square(h2, hh)
                else:
                    nc.vector.tensor_tensor(out=h2, in0=hh, in1=hh,
                                            op=Alu.mult)
                q2 = p2b.tile([80, F], BF16, tag="q2")
                if t % 4 == 1:
                    nc.gpsimd.tensor_mul(q2, h2, mm)
                else:
                    nc.vector.tensor_tensor(out=q2, in0=h2, in1=mm,
                                            op=Alu.mult)
                for hh2 in range(2):
                    sl = slice(hh2 * 512, (hh2 + 1) * 512)
                    nc.tensor.matmul(
                        psI2, sb["kpat_bf"][:80, :], q2[:, sl],
                        start=(t == 0 and hh2 == 0),
                        stop=(t == NT - 1 and hh2 == 1))

            junk5 = singles.tile([K, 512], F32)
            inst5 = singles.tile([K, 1], F32)
            nc.scalar.activation(out=junk5, in_=psI2, func=Act.Copy,
                                 bias=0.0, scale=1.0, accum_out=inst5)
            nc.sync.dma_start(out=o_inst[:].unsqueeze(1), in_=inst5)

    from concourse.library_overlay import lower_extended_insts
    lower_extended_insts(nc)
    _split_multiwait(nc)
    return nc


_NC_CACHE = None


def _get_nc():
    global _NC_CACHE
    if _NC_CACHE is None:
        _NC_CACHE = build_program()
    return _NC_CACHE


def run_device(embedding, maskf, trace=False):
    nc = _get_nc()
    in_maps = [
        {"emb": _to_bf16(np.ascontiguousarray(embedding[b].reshape(D, P))),
         "maskb": _to_bf16(np.ascontiguousarray(maskf[b].reshape(P)))}
        for b in range(B)
    ]
    res = run_bass_kernel_spmd(nc, in_maps, list(range(B)), trace=trace)
    return res


def finalize(per_core):
    """Combine per-image device stats into the 4 reference losses."""
    loss_var_b = np.zeros(B, np.float32)
    loss_dist_b = np.zeros(B, np.float32)
    loss_reg_b = np.zeros(B, np.float32)
    Ns = np.zeros(B, np.float32)
    iu = np.triu(np.ones((K, K), bool), k=1)
    for b in range(B):
        s45 = per_core[b]["o_stats"].astype(np.float32)
        c = per_core[b]["o_c"].astype(np.float32).reshape(K, D)
        inst = per_core[b]["o_inst"].astype(np.float32)
        counts = s45[8::9]
        present = counts > 0
        presentf = present.astype(np.float32)
        N = presentf.sum()
        Ns[b] = N
        inst_mean = inst / np.maximum(counts, 1.0)
        loss_var_b[b] = (inst_mean * presentf).sum() / max(N, 1.0)
        diff = c[:, None, :] - c[None, :, :]
        dist_sq = (diff ** 2).sum(-1)
        pair_mask = present[:, None] & present[None, :] & iu
        safe = np.sqrt(np.where(pair_mask, dist_sq, 1.0))
        term = np.maximum(2.0 * DELTA_D - safe, 0.0) ** 2 * pair_mask
        n_pairs = N * (N - 1.0) / 2.0
        loss_dist_b[b] = term.sum() / (n_pairs if N > 1 else 1.0)
        c_norm = np.sqrt(np.where(present, (c ** 2).sum(-1), 1.0))
        loss_reg_b[b] = (c_norm * presentf).sum() / max(N, 1.0)
    has = (Ns > 0).astype(np.float32)
    denom = max(has.sum(), 1.0)
    loss_var = float((loss_var_b * has).sum() / denom)
    loss_dist = float((loss_dist_b * has).sum() / denom)
    loss_reg = float((loss_reg_b * has).sum() / denom)
    total = ALPHA * loss_var + BETA * loss_dist + GAMMA * loss_reg
    return (np.float32(total), np.float32(loss_var),
            np.float32(loss_dist), np.float32(loss_reg))


def kernel(embedding, instance_mask):
    embedding = np.asarray(embedding, dtype=np.float32)
    maskf = np.asarray(instance_mask).astype(np.float32)
    res = run_device(embedding, maskf, trace=False)
    return finalize(res.results)


# revision 54
# speedup vs baseline: 1.6900x; 1.0017x over previous
"""Discriminative loss kernel for Trainium2 (Bass/Tile), 8-core SPMD.

Data-parallel over batch: core b processes image b (B=8).
Per image, P = 512*1024 pixels, D=8 channels, K=5 labels (0 = background).

pass 1 (segment sums on the PE via pixel-major chunks):
  The flat image [128, 8, 4096] is processed in 4 quarters. Each quarter:
  PE-transpose the 8 e-planes and the label plane into pixel-on-partition
  tiles ([128, 1024] per plane, via 128x128 transposing matmuls through
  PSUM), build one-hot planes ohT[k] with 4x tensor_scalar is_equal, then
  for each 128-pixel chunk accumulate
      psum[k, 0:8] += ohT_chunk^T @ eT_chunk,  psum[k, 8] += sum(ohT_chunk)
  into a single [5, 9] PSUM tile (stationary reloads are pipelined;
  4096 tiny matmuls total). This replaces the DVE masked-product planes,
  the ones-matmul reduction and the ACT accumulate pass of the previous
  version.
tiny device math: centers c = sums/counts, |c_k|^2 bias, and the
  block-diagonal stationary holding -2*c for pass 2 (as before).
pass 2 ((g,d) blocked layout: partition = g*8+d, g=16 pixel groups):
  psum[(g,k),f] = sum_d(-2 c_kd e_d) + |e|^2       (two PE matmuls)
  d = sqrt(psum + C2_k); h = relu(d - 0.5); h2 = h^2
  inst_sum[k] += sum_f h2 * (label==k+1)
  The one-hot arrives via a stride-0 broadcast DMA of the raw labels into
  [80, F] plus a 4x is_equal (no DRAM one-hot round trip).
Host combines the per-image scalars into the final 4 losses.
"""

import os
import sys

import numpy as np

for _p in ("/opt/trn_rl_repo", "/root/.axon_site/_ro/trn_rl_repo"):
    if os.path.isdir(_p) and _p not in sys.path:
        sys.path.insert(0, _p)

import concourse.bass as bass
import concourse.tile as tile
from concourse import mybir
from concourse.bass_utils import run_bass_kernel_spmd

F32 = mybir.dt.float32
BF16 = mybir.dt.bfloat16
F8 = mybir.dt.float8e4
Alu = mybir.AluOpType
Act = mybir.ActivationFunctionType

B, D, H, W = 8, 8, 512, 1024
P = H * W          # 524288 pixels
K = 5
R = 128            # sbuf partitions
COLS = P // R      # 4096 flat cols
NQ = 8             # pass-1 slices
QW = COLS // NQ    # 512 cols per slice
NB = QW // R       # 4 transpose blocks per plane per slice
G = 16             # pass-2 pixel groups
GPP = P // G       # 32768 pixels per group
F = 1024           # pass-2 tile width
NT = GPP // F      # 32 tiles
DELTA_V = 0.5
DELTA_D = 3.0
ALPHA, BETA, GAMMA = 1.0, 1.0, 0.001


def _to_bf16(a):
    import ml_dtypes
    return a.astype(ml_dtypes.bfloat16)


def _to_f8(a):
    import ml_dtypes
    return a.astype(ml_dtypes.float8_e4m3fn)


def _build_consts():
    sel_cnt = np.zeros((R, 40), np.float32)
    sel_sum = np.zeros((R, 40), np.float32)
    for k in range(K):
        for d in range(D):
            sel_cnt[9 * k + 8, 8 * k + d] = 1.0
            sel_sum[9 * k + d, 8 * k + d] = 1.0
    sum5 = np.zeros((R, K), np.float32)
    for k in range(K):
        for d in range(D):
            sum5[8 * k + d, k] = 1.0
    rep80 = np.zeros((R, 80), np.float32)
    for g in range(G):
        for k in range(K):
            rep80[k, 5 * g + k] = 1.0
    smat = np.zeros((R, 80), np.float32)
    for kk in range(K):
        for d in range(D):
            for g in range(G):
                smat[8 * kk + d, 5 * g + kk] = 1.0
    dsel = np.zeros((R, R), np.float32)
    for k in range(K):
        for d in range(D):
            for g in range(G):
                dsel[8 * k + d, 8 * g + d] = 1.0
    blockmask = np.zeros((R, 80), np.float32)
    for g in range(G):
        for d in range(D):
            for k in range(K):
                blockmask[8 * g + d, 5 * g + k] = 1.0
    ones_col = np.ones((R, 1), np.float32)
    kpat = np.zeros((R, K), np.float32)
    kvec = np.zeros((R, 1), np.float32)
    for g in range(G):
        for k in range(K):
            kpat[5 * g + k, k] = 1.0
            kvec[5 * g + k, 0] = float(k + 1)
    scat45 = np.zeros((R, 45), np.float32)
    dmask45 = np.zeros((R, 9), np.float32)
    for k in range(K):
        for dd in range(9):
            scat45[k, 9 * k + dd] = 1.0
            dmask45[9 * k + dd, dd] = 1.0
    f32_pack = dict(sel_cnt=sel_cnt, sel_sum=sel_sum, sum5=sum5,
                    rep80=rep80, smat=smat, dsel=dsel, blockmask=blockmask,
                    ones_col=ones_col, kpat=kpat, kvec=kvec,
                    scat45=scat45, dmask45=dmask45)
    bf_pack = dict(ident_bf=_to_bf16(np.eye(R, dtype=np.float32)),
                   blockmask_bf=_to_bf16(blockmask),
                   kpat_bf=_to_bf16(kpat))
    return f32_pack, bf_pack


def _ap(handle, offset, dims):
    return bass.AP(tensor=handle.tensor if isinstance(handle, bass.AP) else handle,
                   offset=offset, ap=[list(x) for x in dims])


def _tap(t, extra_offset, dims):
    """Sub-AP of an SBUF tile at t.offset + extra_offset."""
    return bass.AP(tensor=t.tensor, offset=t.offset + extra_offset,
                   ap=[list(x) for x in dims])


def _split_multiwait(nc):
    """This container's walrus encodes at most one sync-wait per instruction;
    Tile's tail drain carries one wait per outstanding DMA queue. Hoist the
    extra waits onto single-wait drains inserted just before."""
    n_split = 0
    for blk in nc.m.functions[0].blocks:
        out = []
        changed = False
        for i in blk.instructions:
            si = i.sync_info
            if si is not None and len(si.on_wait) > 1:
                waits = list(si.on_wait)
                for w in waits[:-1]:
                    d = mybir.InstDrain(
                        name=nc.get_next_instruction_name(), ins=[], outs=[])
                    d.engine = i.engine
                    d.sync_info = mybir.SyncInfo(on_wait=[w], on_update=[])
                    out.append(d)
                    n_split += 1
                i.sync_info = mybir.SyncInfo(
                    on_wait=[waits[-1]], on_update=list(si.on_update))
                changed = True
            out.append(i)
        if changed:
            blk.instructions = out
    return n_split


def build_program():
    nc = bass.Bass()
    emb = nc.declare_dram_parameter("emb", [D, P], BF16, isOutput=False)
    maskb = nc.declare_dram_parameter("maskb", [P], BF16, isOutput=False)
    o_stats = nc.declare_dram_parameter("o_stats", [45], F32, isOutput=True)
    o_c = nc.declare_dram_parameter("o_c", [40], F32, isOutput=True)
    o_inst = nc.declare_dram_parameter("o_inst", [K], F32, isOutput=True)

    f32_pack, bf_pack = _build_consts()
    cat_f32 = nc.inline_tensor(
        np.concatenate([v for v in f32_pack.values()], axis=1), name="c_f32")
    cat_bf = nc.inline_tensor(
        np.concatenate([v for v in bf_pack.values()], axis=1), name="c_bf")

    with tile.TileContext(nc) as tc:
        with tc.tile_pool(name="singles", bufs=1) as singles, \
             tc.tile_pool(name="ld", bufs=3) as ldp, \
             tc.tile_pool(name="tr", bufs=3) as trp, \
             tc.tile_pool(name="p2a", bufs=16) as p2a, \
             tc.tile_pool(name="p2l", bufs=4) as p2l, \
             tc.tile_pool(name="p2b", bufs=3) as p2b, \
             tc.tile_pool(name="psumA", bufs=3, space="PSUM") as psumA, \
             tc.tile_pool(name="psumB", bufs=1, space="PSUM") as psumB, \
             tc.tile_pool(name="psum2", bufs=2, space="PSUM") as psum2:

            # first data slice BEFORE the constants: the dozen constant
            # DMAs otherwise serialize ~13us of dead time at the head.
            ebf0 = ldp.tile([R, D, QW], BF16, tag="ebf", name="ebf0")
            lbq0 = ldp.tile([R, QW], BF16, tag="lbq", name="lb0")
            nc.sync.dma_start(
                out=ebf0, in_=_ap(emb, 0, [[COLS, R], [P, D], [1, QW]]))
            nc.sync.dma_start(
                out=lbq0, in_=_ap(maskb, 0, [[COLS, R], [1, QW]]))

            # load constants: two packed DMAs (bf16 first: transposes
            # need the identity), then carve slice views
            sb = {}
            wbf = sum(v.shape[1] for v in bf_pack.values())
            wf32 = sum(v.shape[1] for v in f32_pack.values())
            tbf = singles.tile([R, wbf], BF16, tag="c_bf")
            nc.sync.dma_start(out=tbf, in_=cat_bf[:])
            tf32 = singles.tile([R, wf32], F32, tag="c_f32")
            nc.sync.dma_start(out=tf32, in_=cat_f32[:])
            off = 0
            for name, v in bf_pack.items():
                sb[name] = tbf[:, off:off + v.shape[1]]
                off += v.shape[1]
            off = 0
            for name, v in f32_pack.items():
                sb[name] = tf32[:, off:off + v.shape[1]]
                off += v.shape[1]

            # constants used as activation biases
            for cval in (0.0, -DELTA_V):
                ct = singles.tile([R, 1], F32, tag=f"bias_{cval}")
                nc.vector.memset(ct, cval)
                nc.const_aps.aps[(F32, cval)] = ct[:]

            # ---------------- pass 1 ----------------
            # per quarter: transpose 8 e-planes + label plane to pixel-major,
            # one-hot via 4x is_equal, then 1024 accumulating chunk matmuls.
            ps59 = psumB.tile([K, D + 1], F32, tag="small")

            def p2_pre(t):
                """Center-independent pass-2 tile work (DMA, one-hot, |e|^2).
                Emitted inside pass 1 for the first NQ tiles to fill engine
                slack and shorten the pass-1 -> pass-2 transition."""
                et2 = p2a.tile([R, F], BF16, tag="et2", name=f"et2_{t}")
                nc.sync.dma_start(
                    out=et2, in_=_ap(emb, t * F, [[GPP, G], [P, D], [1, F]]))
                lb2 = p2l.tile([80, F], BF16, tag="lb2", name=f"lb2_{t}")
                nc.sync.dma_start(
                    out=lb2, in_=_ap(maskb, t * F, [[GPP, G], [0, K], [1, F]]))
                mm = p2a.tile([80, F], BF16, tag="mm", name=f"mm_{t}")
                nc.vector.tensor_scalar(
                    out=mm, in0=lb2, scalar1=sb["kvec"][:80, :],
                    scalar2=None, op0=Alu.is_equal)
                sq = p2a.tile([R, F], BF16, tag="sq", name=f"sq_{t}")
                if t % 8 in (0, 4):
                    nc.scalar.square(sq, et2)
                elif t % 8 in (2, 6):
                    nc.vector.tensor_tensor(out=sq, in0=et2, in1=et2,
                                            op=Alu.mult)
                else:
                    nc.gpsimd.tensor_mul(sq, et2, et2)
                return et2, mm, sq

            stage = {}
            pre2 = {}
            for q in range(NQ + 1):
                if q < NQ:
                    base = q * QW
                    if q == 0:
                        ebf, lbq = ebf0, lbq0
                    else:
                        ebf = ldp.tile([R, D, QW], BF16, tag="ebf",
                                       name=f"ebf{q}")
                        lbq = ldp.tile([R, QW], BF16, tag="lbq",
                                       name=f"lb{q}")
                        nc.sync.dma_start(
                            out=ebf,
                            in_=_ap(emb, base, [[COLS, R], [P, D], [1, QW]]))
                        nc.sync.dma_start(
                            out=lbq,
                            in_=_ap(maskb, base, [[COLS, R], [1, QW]]))

                    eT = trp.tile([R, D + 1, QW], BF16, tag="eT",
                                  name=f"eT{q}")
                    ohT = trp.tile([R, K, QW], BF16, tag="ohT",
                                   name=f"ohT{q}")
                    lT = trp.tile([R, QW], BF16, tag="lT", name=f"lT{q}")
                    if q < 3:
                        nc.gpsimd.memset(eT[:, D, :], 1.0)

                    # labels first (one-hots start while e transposes run)
                    pst = psumA.tile([R, QW], BF16, tag="trans")
                    for b in range(NB):
                        nc.tensor.transpose(
                            pst[:, b * R:(b + 1) * R],
                            lbq[:, b * R:(b + 1) * R], sb["ident_bf"])
                    nc.scalar.copy(out=lT, in_=pst)
                    for k in range(K):
                        nc.vector.tensor_scalar(
                            out=ohT[:, k, :], in0=lT, scalar1=float(k + 1),
                            scalar2=None, op0=Alu.is_equal)

                    for d in range(D):
                        pse = psumA.tile([R, QW], BF16, tag="trans",
                                         name=f"pse{q}_{d}")
                        for b in range(NB):
                            nc.tensor.transpose(
                                pse[:, b * R:(b + 1) * R],
                                ebf[:, d, b * R:(b + 1) * R], sb["ident_bf"])
                        # GPSIMD has no PSUM access: ACT/DVE only here
                        if d % 2 == 1:
                            nc.vector.tensor_copy(out=eT[:, d, :], in_=pse)
                        else:
                            nc.scalar.copy(out=eT[:, d, :], in_=pse)
                    stage[q] = (eT, ohT)
                    pre2[q] = p2_pre(q)

                if q >= 1:
                    # chunk matmuls for the PREVIOUS slice: PE runs these
                    # while this slice's copies drain on ACT/DVE/Pool.
                    eTp, ohTp = stage.pop(q - 1)
                    for c in range(QW):
                        nc.tensor.matmul(
                            ps59,
                            _tap(ohTp, c, [[ohTp.ap[0][0], R], [QW, K]]),
                            _tap(eTp, c, [[eTp.ap[0][0], R], [QW, D + 1]]),
                            start=(q == 1 and c == 0),
                            stop=(q == NQ and c == QW - 1))

            for t_ in range(NQ, NQ + 8):
                pre2[t_] = p2_pre(t_)

            # [5, 9] psum -> 45-long stats vector, fully on-chip:
            # scatter matmul to [45, 9], diagonal mask, free-dim reduce.
            # (a DMA rearrange here costs ~3us of all-engine stall)
            s59 = singles.tile([K, D + 1], F32)
            nc.scalar.copy(out=s59, in_=ps59)
            sb45 = singles.tile([R, 1], F32)
            nc.vector.memset(sb45, 0.0)
            ps45x9 = psumB.tile([45, 9], F32, tag="small")
            nc.tensor.matmul(ps45x9, sb["scat45"][:K, :], s59,
                             start=True, stop=True)
            s45x9 = singles.tile([45, 9], F32)
            nc.vector.tensor_tensor(out=s45x9, in0=ps45x9,
                                    in1=sb["dmask45"][:45, :], op=Alu.mult)
            nc.vector.tensor_reduce(
                out=sb45[:45, :], in_=s45x9,
                axis=mybir.AxisListType.X, op=Alu.add)
            nc.sync.dma_start(out=o_stats[:].unsqueeze(1), in_=sb45[:45, :])

            # ---------------- tiny math: centers ----------------
            ps40a = psumB.tile([40, 1], F32, tag="small")
            nc.tensor.matmul(ps40a, sb["sel_cnt"], sb45, start=True, stop=True)
            ps40b = psumB.tile([40, 1], F32, tag="small")
            nc.tensor.matmul(ps40b, sb["sel_sum"], sb45, start=True, stop=True)
            cntc = singles.tile([R, 1], F32)
            nc.vector.memset(cntc, 0.0)
            nc.vector.tensor_scalar(out=cntc[:40, :], in0=ps40a, scalar1=1.0,
                                    scalar2=None, op0=Alu.max)
            inv = singles.tile([R, 1], F32)
            nc.vector.memset(inv, 0.0)
            nc.vector.reciprocal(out=inv[:40, :], in_=cntc[:40, :])
            c40 = singles.tile([R, 1], F32)
            nc.vector.memset(c40, 0.0)
            nc.vector.tensor_tensor(out=c40[:40, :], in0=ps40b, in1=inv[:40, :],
                                    op=Alu.mult)
            nc.sync.dma_start(out=o_c[:].unsqueeze(1), in_=c40[:40, :])
            cm2 = singles.tile([R, 1], F32)
            nc.vector.memset(cm2, 0.0)
            nc.vector.tensor_scalar(out=cm2[:40, :], in0=c40[:40, :],
                                    scalar1=-2.0, scalar2=None, op0=Alu.mult)
            csq = singles.tile([R, 1], F32)
            nc.vector.memset(csq, 0.0)
            nc.vector.tensor_tensor(out=csq[:40, :], in0=c40[:40, :],
                                    in1=c40[:40, :], op=Alu.mult)
            ps5 = psumB.tile([K, 1], F32, tag="small")
            nc.tensor.matmul(ps5, sb["sum5"], csq, start=True, stop=True)
            c2sb = singles.tile([R, 1], F32)
            nc.vector.memset(c2sb, 0.0)
            nc.scalar.copy(out=c2sb[:K, :], in_=ps5)
            ps80 = psumB.tile([80, 1], F32, tag="small")
            nc.tensor.matmul(ps80, sb["rep80"], c2sb, start=True, stop=True)
            c2bias = singles.tile([R, 1], F32)
            nc.vector.memset(c2bias, 0.0)
            nc.scalar.copy(out=c2bias[:80, :], in_=ps80)

            # block-diagonal stationary: cblk[8g+d, 5g+k] = -2*c[k,d]
            rhsS = singles.tile([R, 80], F32)
            nc.vector.tensor_scalar(out=rhsS, in0=sb["smat"], scalar1=cm2,
                                    scalar2=None, op0=Alu.mult)
            psD = psumB.tile([R, 80], F32, tag="small")
            nc.tensor.matmul(psD, sb["dsel"], rhsS, start=True, stop=True)
            cblk_bf = singles.tile([R, 80], BF16)
            nc.vector.tensor_tensor(out=cblk_bf, in0=psD,
                                    in1=sb["blockmask"], op=Alu.mult)

            # ---------------- pass 2 ----------------
            psI2 = psumB.tile([K, 512], F32, tag="small")
            for t in range(NT):
                if t in pre2:
                    et2, mm, sq = pre2.pop(t)
                else:
                    et2, mm, sq = p2_pre(t)
                pt = psum2.tile([80, F], F32, tag="pt")
                for hh_ in range(2):
                    sl = slice(hh_ * 512, (hh_ + 1) * 512)
                    nc.tensor.matmul(pt[:, sl], cblk_bf, et2[:, sl],
                                     start=True, stop=False)
                    nc.tensor.matmul(pt[:, sl], sb["blockmask_bf"], sq[:, sl],
                                     start=False, stop=True)
                dd = p2b.tile([80, F], BF16, tag="dd")
                nc.scalar.activation(out=dd, in_=pt, func=Act.Sqrt,
                                     bias=c2bias[:80, :], scale=1.0)
                hh = p2b.tile([80, F], BF16, tag="hh")
                nc.vector.tensor_scalar(out=hh, in0=dd, scalar1=-DELTA_V,
                                        scalar2=0.0, op0=Alu.add, op1=Alu.max)
                h2 = p2b.tile([80, F], BF16, tag="h2")
                if t % 4 == 3:
                    nc.scalar.square(h2, hh)
                else:
                    nc.vector.tensor_tensor(out=h2, in0=hh, in1=hh,
                                            op=Alu.mult)
                q2 = p2b.tile([80, F], BF16, tag="q2")
                if t % 4 == 1:
                    nc.gpsimd.tensor_mul(q2, h2, mm)
                else:
                    nc.vector.tensor_tensor(out=q2, in0=h2, in1=mm,
                                            op=Alu.mult)
                for hh2 in range(2):
                    sl = slice(hh2 * 512, (hh2 + 1) * 512)
                    nc.tensor.matmul(
                        psI2, sb["kpat_bf"][:80, :], q2[:, sl],
                        start=(t == 0 and hh2 == 0),
                        stop=(t == NT - 1 and hh2 == 1))

            junk5 = singles.tile([K, 512], F32)
            inst5 = singles.tile([K, 1], F32)
            nc.scalar.activation(out=junk5, in_=psI2, func=Act.Copy,
                                 bias=0.0, scale=1.0, accum_out=inst5)
            nc.sync.dma_start(out=o_inst[:].unsqueeze(1), in_=inst5)

    from concourse.library_overlay import lower_extended_insts
    lower_extended_insts(nc)
    _split_multiwait(nc)
    return nc


_NC_CACHE = None


def _get_nc():
    global _NC_CACHE
    if _NC_CACHE is None:
        _NC_CACHE = build_program()
    return _NC_CACHE


def run_device(embedding, maskf, trace=False):
    nc = _get_nc()
    in_maps = [
        {"emb": _to_bf16(np.ascontiguousarray(embedding[b].reshape(D, P))),
         "maskb": _to_bf16(np.ascontiguousarray(maskf[b].reshape(P)))}
        for b in range(B)
    ]
    res = run_bass_kernel_spmd(nc, in_maps, list(range(B)), trace=trace)
    return res


def finalize(per_core):
    """Combine per-image device stats into the 4 reference losses."""
    loss_var_b = np.zeros(B, np.float32)
    loss_dist_b = np.zeros(B, np.float32)
    loss_reg_b = np.zeros(B, np.float32)
    Ns = np.zeros(B, np.float32)
    iu = np.triu(np.ones((K, K), bool), k=1)
    for b in range(B):
        s45 = per_core[b]["o_stats"].astype(np.float32)
        c = per_core[b]["o_c"].astype(np.float32).reshape(K, D)
        inst = per_core[b]["o_inst"].astype(np.float32)
        counts = s45[8::9]
        present = counts > 0
        presentf = present.astype(np.float32)
        N = presentf.sum()
        Ns[b] = N
        inst_mean = inst / np.maximum(counts, 1.0)
        loss_var_b[b] = (inst_mean * presentf).sum() / max(N, 1.0)
        diff = c[:, None, :] - c[None, :, :]
        dist_sq = (diff ** 2).sum(-1)
        pair_mask = present[:, None] & present[None, :] & iu
        safe = np.sqrt(np.where(pair_mask, dist_sq, 1.0))
        term = np.maximum(2.0 * DELTA_D - safe, 0.0) ** 2 * pair_mask
        n_pairs = N * (N - 1.0) / 2.0
        loss_dist_b[b] = term.sum() / (n_pairs if N > 1 else 1.0)
        c_norm = np.sqrt(np.where(present, (c ** 2).sum(-1), 1.0))
        loss_reg_b[b] = (c_norm * presentf).sum() / max(N, 1.0)
    has = (Ns > 0).astype(np.float32)
    denom = max(has.sum(), 1.0)
    loss_var = float((loss_var_b * has).sum() / denom)
    loss_dist = float((loss_dist_b * has).sum() / denom)
    loss_reg = float((loss_reg_b * has).sum() / denom)
    total = ALPHA * loss_var + BETA * loss_dist + GAMMA * loss_reg
    return (np.float32(total), np.float32(loss_var),
            np.float32(loss_dist), np.float32(loss_reg))


def kernel(embedding, instance_mask):
    embedding = np.asarray(embedding, dtype=np.float32)
    maskf = np.asarray(instance_mask).astype(np.float32)
    res = run_device(embedding, maskf, trace=False)
    return finalize(res.results)
